# revision 2
# baseline (speedup 1.0000x reference)
"""Trainium2 Bass kernel for nn_Encoder (sliding-window MLP + synaptic conv).

Computation (per timestep t of T_data):
  syn_e[t] = sum(S_e[t, :]);  syn_i[t] = sum(S_i[t, :])
  syn_out[t, s] = sum_k e_kern[s, k] * syn_e[t-k] + i_kern[s, k] * syn_i[t-k]
  Vw[t, :] = V[t-199 : t+200]   (zero padded)
  h = lrelu(Vw @ W1.T + b1); h = lrelu(h @ W2.T + b2); h = lrelu(h @ W3.T + b3)
  out[t, :] = tanh(h @ W4.T + b4 + syn_out[t, :])

Strategy: data-parallel over T across 8 NeuronCores, each core gets its
T/8 slice plus a 199-row halo of S_e/S_i and a 398-elem halo of V (host
zero-pads the edges).  On each core:
  - S_e/S_i row-sums via VectorE free-axis reduce (fp32), cast to bf16,
    scattered to a DRAM scratch vector.
  - All matmuls in bf16 (fp32 PSUM accumulate).  Sliding windows of V and
    syn_e/syn_i are materialized as matmul operands directly by overlapping
    ("Hankel") DMA access patterns from DRAM: partition step 1, free step 1.
  - Layers 1-3 keep hid on PSUM partitions; layer 4 + conv are computed
    transposed (t on partitions) so the output DMA is contiguous; b4 is
    added via a K=1 ones-matmul into the same PSUM accumulation group.
  - lrelu/tanh run on ScalarE (Lrelu alpha=0.01, Tanh); both live in the
    same ACT LUT table set so only one table load is emitted.
"""

import os
from contextlib import ExitStack

import ml_dtypes
import numpy as np

import concourse.bass as bass
import concourse.mybir as mybir
import concourse.tile as tile
from concourse import bacc
from concourse.bass_utils import run_bass_kernel_spmd

BF16 = ml_dtypes.bfloat16
FP32 = mybir.dt.float32
BF = mybir.dt.bfloat16

T_NO = 200
WIN = 2 * T_NO - 1  # 399
N_CORES = 8
BLK = 512  # timesteps per block (one PSUM bank of fp32)

LAST = {}  # exec_time_ns / trace info from the most recent run (for test harness)


def _ceil_div(a, b):
    return -(-a // b)


def _chunks(total, step=128):
    out = []
    o = 0
    while o < total:
        out.append((o, min(step, total - o)))
        o += step
    return out


def _build(T_PAD, SE_ROWS, E_COLS, I_COLS, HID, SUB):
    """Build the per-core Bass program (identical on all 8 cores)."""
    R_TILES = _ceil_div(SE_ROWS, 128)
    SCR_LEN = R_TILES * 128  # scratch syn vectors, padded to reduce tiles
    V_LEN = T_PAD + WIN - 1
    NB = _ceil_div(T_PAD, BLK)

    m_hid = _chunks(HID)  # M chunks of hid (PSUM partitions L1-3)
    k_win = _chunks(WIN)  # K chunks of the V window
    k_hid = _chunks(HID)  # K chunks of hid (L2-4 contraction)
    k_syn = _chunks(T_NO)  # K chunks of the conv kernel length

    nc = bacc.Bacc(
        "TRN2", target_bir_lowering=False, debug=False, num_devices=N_CORES
    )

    se_h = nc.dram_tensor("se", [SE_ROWS, E_COLS], FP32, kind="ExternalInput")
    si_h = nc.dram_tensor("si", [SE_ROWS, I_COLS], FP32, kind="ExternalInput")
    v_h = nc.dram_tensor("v", [V_LEN], BF, kind="ExternalInput")
    w1t_h = nc.dram_tensor("w1t", [WIN, HID], BF, kind="ExternalInput")
    w2t_h = nc.dram_tensor("w2t", [HID, HID], BF, kind="ExternalInput")
    w3t_h = nc.dram_tensor("w3t", [HID, HID], BF, kind="ExternalInput")
    w4t_h = nc.dram_tensor("w4t", [HID, SUB], BF, kind="ExternalInput")
    b1_h = nc.dram_tensor("b1", [HID], FP32, kind="ExternalInput")
    b2_h = nc.dram_tensor("b2", [HID], FP32, kind="ExternalInput")
    b3_h = nc.dram_tensor("b3", [HID], FP32, kind="ExternalInput")
    b4_h = nc.dram_tensor("b4", [SUB], BF, kind="ExternalInput")
    ekm_h = nc.dram_tensor("ekm", [T_NO, SUB], BF, kind="ExternalInput")
    ikm_h = nc.dram_tensor("ikm", [T_NO, SUB], BF, kind="ExternalInput")
    out_h = nc.dram_tensor("out", [T_PAD, SUB], FP32, kind="ExternalOutput")

    sse_h = nc.dram_tensor("sse_scratch", [SCR_LEN], BF)
    ssi_h = nc.dram_tensor("ssi_scratch", [SCR_LEN], BF)

    with tile.TileContext(nc) as tc, ExitStack() as ctx:
        cpool = ctx.enter_context(tc.tile_pool(name="consts", bufs=1))
        sepool = ctx.enter_context(tc.tile_pool(name="sein", bufs=4))
        accpool = ctx.enter_context(tc.tile_pool(name="acc", bufs=1))
        stpool = ctx.enter_context(tc.tile_pool(name="store", bufs=2))
        hkpool = ctx.enter_context(tc.tile_pool(name="hankel", bufs=2))
        hpool = ctx.enter_context(tc.tile_pool(name="acts", bufs=2))
        opool = ctx.enter_context(tc.tile_pool(name="outs", bufs=3))
        psmm = ctx.enter_context(tc.tile_pool(name="psmm", bufs=6, space="PSUM"))
        ps4p = ctx.enter_context(tc.tile_pool(name="ps4p", bufs=2, space="PSUM"))

        # ---- constants to SBUF ----
        def load_rows(dram, rows, cols, dt, nm):
            tiles = []
            for j, (o, p) in enumerate(rows):
                t = cpool.tile([128, cols], dt, name=f"{nm}{j}", tag=f"{nm}{j}")
                nc.sync.dma_start(out=t[:p, :], in_=dram[o : o + p, :])
                tiles.append(t)
            return tiles

        w1t_sb = load_rows(w1t_h, k_win, HID, BF, "w1t")
        w2t_sb = load_rows(w2t_h, k_hid, HID, BF, "w2t")
        w3t_sb = load_rows(w3t_h, k_hid, HID, BF, "w3t")
        w4t_sb = load_rows(w4t_h, k_hid, SUB, BF, "w4t")
        ek_sb = load_rows(ekm_h, k_syn, SUB, BF, "ek")
        ik_sb = load_rows(ikm_h, k_syn, SUB, BF, "ik")

        bias_sb = {}
        for nm, h in (("b1", b1_h), ("b2", b2_h), ("b3", b3_h)):
            t = cpool.tile([128, len(m_hid)], FP32, name=nm, tag=nm)
            for c, (o, p) in enumerate(m_hid):
                nc.sync.dma_start(out=t[:p, c], in_=h[o : o + p])
            bias_sb[nm] = t

        b4_sb = cpool.tile([1, SUB], BF, name="b4sb", tag="b4sb")
        nc.sync.dma_start(out=b4_sb[0:1, :], in_=bass.AP(b4_h, 0, [[0, 1], [1, SUB]]))
        ones_sb = cpool.tile([1, 128], BF, name="ones", tag="ones")
        nc.vector.memset(ones_sb[0:1, :], 1.0)

        # ---- reduction accumulators ----
        se_acc = accpool.tile([128, R_TILES], FP32, name="se_acc", tag="se_acc")
        si_acc = accpool.tile([128, R_TILES], FP32, name="si_acc", tag="si_acc")
        if SE_ROWS % 128 != 0:
            # rows past SE_ROWS in the last reduce tile are never written by
            # the reduce; zero them so the scratch tail holds no garbage
            nc.vector.memset(se_acc[:, R_TILES - 1 : R_TILES], 0.0)
            nc.vector.memset(si_acc[:, R_TILES - 1 : R_TILES], 0.0)

        reduced = 0  # reduce tiles emitted so far
        stored = 0  # scratch columns stored so far

        def emit_reduce(i):
            r0 = 128 * i
            nr = min(128, SE_ROWS - r0)
            se_t = sepool.tile([128, E_COLS], FP32, name="se_t", tag="se_t")
            nc.sync.dma_start(out=se_t[:nr, :], in_=se_h[r0 : r0 + nr, :])
            nc.vector.reduce_sum(se_acc[:nr, i : i + 1], se_t[:nr, :],
                                 axis=mybir.AxisListType.X)
            si_t = sepool.tile([128, I_COLS], FP32, name="si_t", tag="si_t")
            nc.sync.dma_start(out=si_t[:nr, :], in_=si_h[r0 : r0 + nr, :])
            nc.vector.reduce_sum(si_acc[:nr, i : i + 1], si_t[:nr, :],
                                 axis=mybir.AxisListType.X)

        def emit_store(a, b):
            # cast fp32 accumulator cols [a,b) to bf16 and scatter to scratch
            w = b - a
            for nm, acc, scr in (("se", se_acc, sse_h), ("si", si_acc, ssi_h)):
                bf_t = stpool.tile([128, w], BF, name=f"{nm}bf", tag=f"{nm}bf",
                                   padded_shape=[128, 8])
                nc.vector.tensor_copy(bf_t[:, :w], acc[:, a:b])
                nc.sync.dma_start(
                    out=bass.AP(scr, 128 * a, [[1, 128], [128, w]]),
                    in_=bf_t[:, :w],
                )

        # ---- main loop over timestep blocks ----
        for bidx in range(NB):
            t0 = BLK * bidx
            nt = min(BLK, T_PAD - t0)
            nsub = _ceil_div(nt, 128)

            # reduce tiles + scratch stores needed by this block's conv reads
            need = min(R_TILES, _ceil_div(t0 + nt + T_NO - 1, 128))
            if bidx == NB - 1:
                need = R_TILES
            while reduced < need:
                emit_reduce(reduced)
                reduced += 1
            if need > stored:
                emit_store(stored, need)
                stored = need

            # Hankel (sliding window) operand tiles
            vh = []
            for j, (o, p) in enumerate(k_win):
                t = hkpool.tile([128, BLK], BF, name=f"vh{j}", tag=f"vh{j}")
                nc.sync.dma_start(
                    out=t[:p, :nt], in_=bass.AP(v_h, t0 + o, [[1, p], [1, nt]])
                )
                vh.append(t)
            synh = {}
            for nm, scr in (("se", sse_h), ("si", ssi_h)):
                lst = []
                for j, (o, p) in enumerate(k_syn):
                    t = hkpool.tile([128, BLK], BF, name=f"{nm}h{j}", tag=f"{nm}h{j}")
                    nc.sync.dma_start(
                        out=t[:p, :nt], in_=bass.AP(scr, t0 + o, [[1, p], [1, nt]])
                    )
                    lst.append(t)
                synh[nm] = lst

            # layers 1..3 (hid on PSUM partitions)
            def layer(rhs_tiles, k_list, w_sb, bias_t, out_nm):
                outs = []
                for mc, (mo, nm_) in enumerate(m_hid):
                    ps = psmm.tile([128, BLK], FP32, name="ps", tag="ps")
                    for kc, (ko, pk) in enumerate(k_list):
                        nc.tensor.matmul(
                            ps[:nm_, :nt],
                            w_sb[kc][:pk, mo : mo + nm_],
                            rhs_tiles[kc][:pk, :nt],
                            start=(kc == 0),
                            stop=(kc == len(k_list) - 1),
                        )
                    h_t = hpool.tile([128, BLK], BF, name=f"{out_nm}{mc}",
                                     tag=f"{out_nm}{mc}")
                    nc.scalar.activation(
                        h_t[:nm_, :nt], ps[:nm_, :nt],
                        mybir.ActivationFunctionType.Lrelu,
                        bias=bias_t[:nm_, mc : mc + 1], alpha=0.01,
                    )
                    outs.append(h_t)
                return outs

            h1 = layer(vh, k_win, w1t_sb, bias_sb["b1"], f"h1_")
            h2 = layer(h1, k_hid, w2t_sb, bias_sb["b2"], f"h2_")
            h3 = layer(h2, k_hid, w3t_sb, bias_sb["b3"], f"h3_")

            # layer 4 + conv, transposed (t on partitions), bias via ones-matmul
            outb = opool.tile([128, nsub, SUB], FP32, name="outb", tag="outb")
            for st in range(nsub):
                ms = st * 128
                ps4 = ps4p.tile([128, SUB], FP32, name="ps4", tag="ps4")
                nc.tensor.matmul(ps4[:, :], ones_sb[0:1, 0:128], b4_sb[0:1, :],
                                 start=True, stop=False)
                for kc, (ko, pk) in enumerate(k_hid):
                    nc.tensor.matmul(
                        ps4[:, :], h3[kc][:pk, ms : ms + 128], w4t_sb[kc][:pk, :],
                        start=False, stop=False,
                    )
                for nm, k_sb in (("se", ek_sb), ("si", ik_sb)):
                    for j, (o, pk) in enumerate(k_syn):
                        last = nm == "si" and j == len(k_syn) - 1
                        nc.tensor.matmul(
                            ps4[:, :], synh[nm][j][:pk, ms : ms + 128],
                            k_sb[j][:pk, :],
                            start=False, stop=last,
                        )
                nc.scalar.activation(outb[:, st, :], ps4[:, :],
                                     mybir.ActivationFunctionType.Tanh)

            nc.sync.dma_start(
                out=bass.AP(out_h, t0 * SUB, [[SUB, 128], [128 * SUB, nsub], [1, SUB]]),
                in_=outb[:, :nsub, :],
            )

    nc.compile()
    return nc


def kernel(V, S_e, S_i, W1, b1, W2, b2, W3, b3, W4, b4, W_syn, Tau_syn, Delta_syn):
    V = np.asarray(V, np.float32)
    S_e = np.ascontiguousarray(np.asarray(S_e, np.float32))
    S_i = np.ascontiguousarray(np.asarray(S_i, np.float32))
    T = V.shape[0]
    assert T % N_CORES == 0
    T_LOC = T // N_CORES
    T_PAD = _ceil_div(T_LOC, 128) * 128
    SE_ROWS = T_NO - 1 + T_LOC
    V_LEN = T_PAD + WIN - 1
    HID = W1.shape[0]
    SUB = W4.shape[0]

    # ---- tiny host-side prep (layout/dtype only + 20x200 conv kernels) ----
    W_syn = np.asarray(W_syn, np.float32)
    Tau_syn = np.asarray(Tau_syn, np.float32)
    Delta_syn = np.asarray(Delta_syn, np.float32)
    t_raw = np.arange(T_NO, dtype=np.float32)[None, :]
    t_e = np.maximum(t_raw - Delta_syn[:, 0:1], 0.0)
    t_i = np.maximum(t_raw - Delta_syn[:, 1:2], 0.0)
    tt_e = t_e / Tau_syn[:, 0:1] ** 2
    tt_i = t_i / Tau_syn[:, 1:2] ** 2
    e_kern = tt_e * np.exp(-tt_e) * W_syn[:, 0:1] ** 2
    i_kern = -(tt_i * np.exp(-tt_i)) * W_syn[:, 1:2] ** 2
    ekm = np.ascontiguousarray(e_kern[:, ::-1].T).astype(BF16)  # [T_NO, SUB]
    ikm = np.ascontiguousarray(i_kern[:, ::-1].T).astype(BF16)

    wd = {
        "w1t": np.ascontiguousarray(np.asarray(W1, np.float32).T).astype(BF16),
        "w2t": np.ascontiguousarray(np.asarray(W2, np.float32).T).astype(BF16),
        "w3t": np.ascontiguousarray(np.asarray(W3, np.float32).T).astype(BF16),
        "w4t": np.ascontiguousarray(np.asarray(W4, np.float32).T).astype(BF16),
        "b1": np.asarray(b1, np.float32),
        "b2": np.asarray(b2, np.float32),
        "b3": np.asarray(b3, np.float32),
        "b4": np.asarray(b4, np.float32).astype(BF16),
        "ekm": ekm,
        "ikm": ikm,
    }

    vg = np.zeros(T_NO - 1 + T + WIN + T_PAD - T_LOC, np.float32)
    vg[T_NO - 1 : T_NO - 1 + T] = V
    vg = vg.astype(BF16)

    halo = T_NO - 1
    ez = np.zeros((halo, S_e.shape[1]), np.float32)
    iz = np.zeros((halo, S_i.shape[1]), np.float32)
    in_maps = []
    for m in range(N_CORES):
        r0 = m * T_LOC
        if m == 0:
            se_m = np.concatenate([ez, S_e[:T_LOC]], 0)
            si_m = np.concatenate([iz, S_i[:T_LOC]], 0)
        else:
            se_m = S_e[r0 - halo : r0 + T_LOC]
            si_m = S_i[r0 - halo : r0 + T_LOC]
        in_maps.append(
            {"se": se_m, "si": si_m, "v": vg[r0 : r0 + V_LEN], **wd}
        )

    nc = _build(T_PAD, SE_ROWS, S_e.shape[1], S_i.shape[1], HID, SUB)
    trace = os.environ.get("CC_TRACE") == "1"
    res = run_bass_kernel_spmd(nc, in_maps, list(range(N_CORES)), trace=trace)
    LAST["exec_time_ns"] = res.exec_time_ns
    LAST["results"] = res
    out = np.concatenate([res.results[m]["out"][:T_LOC] for m in range(N_CORES)], 0)
    return np.ascontiguousarray(out.astype(np.float32))


# revision 5
# speedup vs baseline: 1.8324x; 1.8324x over previous
"""Trainium2 Bass kernel for nn_Encoder (sliding-window MLP + synaptic conv).

Computation (per timestep t of T_data):
  syn_e[t] = sum(S_e[t, :]);  syn_i[t] = sum(S_i[t, :])
  syn_out[t, s] = sum_k e_kern[s, k] * syn_e[t-k] + i_kern[s, k] * syn_i[t-k]
  Vw[t, :] = V[t-199 : t+200]   (zero padded)
  h = lrelu(Vw @ W1.T + b1); h = lrelu(h @ W2.T + b2); h = lrelu(h @ W3.T + b3)
  out[t, :] = tanh(h @ W4.T + b4 + syn_out[t, :])

Strategy: data-parallel over T across 8 NeuronCores, each core gets its
T/8 slice plus a 199-row halo of S_e/S_i and a 398-elem halo of V (host
zero-pads the edges).  On each core:
  - S_e/S_i row-sums via VectorE free-axis reduce (fp32), PE-transposed and
    cast to bf16, stored contiguously to a DRAM scratch vector.
  - All matmuls in bf16 (fp32 PSUM accumulate).  Sliding windows of V and
    syn_e/syn_i are materialized as matmul operands directly by overlapping
    ("Hankel") DMA access patterns from DRAM: partition step 1, free step 1.
    One wide Hankel tile per block serves all K-chunks as column slices.
  - Layers 1-3 keep hid on PSUM partitions; layer 4 + conv keep the
    [sub, t] orientation (weights stationary) so the output store is a
    contiguous [sub, nt] tile; the host transposes the gathered output.
    b4 is added via a K=1 matmul (b4 stationary, ones streaming).
  - lrelu/tanh run on ScalarE (Lrelu alpha=0.01, Tanh); both live in the
    same ACT LUT table set so only one table load is emitted.
"""

import os
from contextlib import ExitStack

import ml_dtypes
import numpy as np

import concourse.bass as bass
import concourse.mybir as mybir
import concourse.tile as tile
from concourse import bacc
from concourse.bass_utils import run_bass_kernel_spmd
from concourse.masks import make_identity

BF16 = ml_dtypes.bfloat16
FP32 = mybir.dt.float32
BF = mybir.dt.bfloat16

T_NO = 200
WIN = 2 * T_NO - 1  # 399
N_CORES = 8
BLK = 512  # timesteps per block (one PSUM bank of fp32)

LAST = {}  # exec_time_ns / trace info from the most recent run (for test harness)


def _ceil_div(a, b):
    return -(-a // b)


def _chunks(total, step=128):
    out = []
    o = 0
    while o < total:
        out.append((o, min(step, total - o)))
        o += step
    return out


def _build(T_PAD, SE_ROWS, E_COLS, I_COLS, HID, SUB):
    """Build the per-core Bass program (identical on all 8 cores)."""
    R_TILES = _ceil_div(SE_ROWS, 128)
    # +128 margin: the wide hankel DMA loads a full [128, nt+W] rectangle
    # whose unused corner reads past the logical end
    SCR_LEN = R_TILES * 128 + 128
    V_LEN = T_PAD + WIN - 1 + 128
    NB = _ceil_div(T_PAD, BLK)

    m_hid = _chunks(HID)  # M chunks of hid (PSUM partitions L1-3)
    k_win = _chunks(WIN)  # K chunks of the V window
    k_hid = _chunks(HID)  # K chunks of hid (L2-4 contraction)
    k_syn = _chunks(T_NO)  # K chunks of the conv kernel length
    VH_W = 128 * (len(k_win) - 1)  # extra hankel cols so K-chunks are slices
    SY_W = 128 * (len(k_syn) - 1)

    nc = bacc.Bacc(
        "TRN2", target_bir_lowering=False, debug=False, num_devices=N_CORES
    )

    se_h = nc.dram_tensor("se", [SE_ROWS, E_COLS], FP32, kind="ExternalInput")
    si_h = nc.dram_tensor("si", [SE_ROWS, I_COLS], FP32, kind="ExternalInput")
    v_h = nc.dram_tensor("v", [V_LEN], BF, kind="ExternalInput")
    w1t_h = nc.dram_tensor("w1t", [WIN, HID], BF, kind="ExternalInput")
    w2t_h = nc.dram_tensor("w2t", [HID, HID], BF, kind="ExternalInput")
    w3t_h = nc.dram_tensor("w3t", [HID, HID], BF, kind="ExternalInput")
    w4t_h = nc.dram_tensor("w4t", [HID, SUB], BF, kind="ExternalInput")
    b1_h = nc.dram_tensor("b1", [HID], FP32, kind="ExternalInput")
    b2_h = nc.dram_tensor("b2", [HID], FP32, kind="ExternalInput")
    b3_h = nc.dram_tensor("b3", [HID], FP32, kind="ExternalInput")
    b4_h = nc.dram_tensor("b4", [SUB], BF, kind="ExternalInput")
    ekm_h = nc.dram_tensor("ekm", [T_NO, SUB], BF, kind="ExternalInput")
    ikm_h = nc.dram_tensor("ikm", [T_NO, SUB], BF, kind="ExternalInput")
    out_h = nc.dram_tensor("out", [SUB, T_PAD], FP32, kind="ExternalOutput")

    sse_h = nc.dram_tensor("sse_scratch", [SCR_LEN], BF)
    ssi_h = nc.dram_tensor("ssi_scratch", [SCR_LEN], BF)

    with tile.TileContext(nc) as tc, ExitStack() as ctx:
        cpool = ctx.enter_context(tc.tile_pool(name="consts", bufs=1))
        sepool = ctx.enter_context(tc.tile_pool(name="sein", bufs=4))
        accpool = ctx.enter_context(tc.tile_pool(name="acc", bufs=1))
        stpool = ctx.enter_context(tc.tile_pool(name="store", bufs=2))
        hkpool = ctx.enter_context(tc.tile_pool(name="hankel", bufs=2))
        hpool = ctx.enter_context(tc.tile_pool(name="acts", bufs=2))
        opool = ctx.enter_context(tc.tile_pool(name="outs", bufs=3))
        psmm = ctx.enter_context(tc.tile_pool(name="psmm", bufs=5, space="PSUM"))
        ps4p = ctx.enter_context(tc.tile_pool(name="ps4p", bufs=2, space="PSUM"))
        ptrp = ctx.enter_context(tc.tile_pool(name="ptrp", bufs=1, space="PSUM"))

        # ---- constants to SBUF ----
        def load_rows(dram, rows, cols, dt, nm):
            tiles = []
            for j, (o, p) in enumerate(rows):
                t = cpool.tile([128, cols], dt, name=f"{nm}{j}", tag=f"{nm}{j}")
                nc.sync.dma_start(out=t[:p, :], in_=dram[o : o + p, :])
                tiles.append(t)
            return tiles

        w1t_sb = load_rows(w1t_h, k_win, HID, BF, "w1t")
        w2t_sb = load_rows(w2t_h, k_hid, HID, BF, "w2t")
        w3t_sb = load_rows(w3t_h, k_hid, HID, BF, "w3t")
        w4t_sb = load_rows(w4t_h, k_hid, SUB, BF, "w4t")
        ek_sb = load_rows(ekm_h, k_syn, SUB, BF, "ek")
        ik_sb = load_rows(ikm_h, k_syn, SUB, BF, "ik")

        bias_sb = {}
        for nm, h in (("b1", b1_h), ("b2", b2_h), ("b3", b3_h)):
            t = cpool.tile([128, len(m_hid)], FP32, name=nm, tag=nm)
            for c, (o, p) in enumerate(m_hid):
                nc.sync.dma_start(out=t[:p, c], in_=h[o : o + p])
            bias_sb[nm] = t

        b4_sb = cpool.tile([1, SUB], BF, name="b4sb", tag="b4sb")
        nc.sync.dma_start(out=b4_sb[0:1, :], in_=bass.AP(b4_h, 0, [[0, 1], [1, SUB]]))
        ones_sb = cpool.tile([1, BLK], BF, name="ones", tag="ones")
        nc.vector.memset(ones_sb[0:1, :], 1.0)
        ident = cpool.tile([128, 128], FP32, name="ident", tag="ident")
        make_identity(nc, ident[:, :])

        # ---- reduction accumulators ----
        se_acc = accpool.tile([128, R_TILES], FP32, name="se_acc", tag="se_acc")
        si_acc = accpool.tile([128, R_TILES], FP32, name="si_acc", tag="si_acc")
        if SE_ROWS % 128 != 0:
            # rows past SE_ROWS in the last reduce tile are never written by
            # the reduce; zero them so the scratch tail holds no garbage
            nc.vector.memset(se_acc[:, R_TILES - 1 : R_TILES], 0.0)
            nc.vector.memset(si_acc[:, R_TILES - 1 : R_TILES], 0.0)

        reduced = 0  # reduce tiles emitted so far
        stored = 0  # scratch columns stored so far

        def emit_reduce(i):
            r0 = 128 * i
            nr = min(128, SE_ROWS - r0)
            se_t = sepool.tile([128, E_COLS], FP32, name="se_t", tag="se_t")
            nc.sync.dma_start(out=se_t[:nr, :], in_=se_h[r0 : r0 + nr, :])
            nc.vector.reduce_sum(se_acc[:nr, i : i + 1], se_t[:nr, :],
                                 axis=mybir.AxisListType.X)
            si_t = sepool.tile([128, I_COLS], FP32, name="si_t", tag="si_t")
            nc.sync.dma_start(out=si_t[:nr, :], in_=si_h[r0 : r0 + nr, :])
            nc.vector.reduce_sum(si_acc[:nr, i : i + 1], si_t[:nr, :],
                                 axis=mybir.AxisListType.X)

        def emit_store(a, b):
            # PE-transpose fp32 accumulator cols [a,b) to [w,128], cast to
            # bf16 on ScalarE, store contiguously to the scratch vector
            w = b - a
            for nm, acc, scr in (("se", se_acc, sse_h), ("si", si_acc, ssi_h)):
                tr_t = ptrp.tile([16, 128], FP32, name=f"{nm}tr", tag="tr")
                nc.tensor.transpose(tr_t[:w, :], acc[:, a:b], ident[:, :])
                st_t = stpool.tile([16, 128], BF, name=f"{nm}st", tag=f"{nm}st")
                nc.scalar.activation(st_t[:w, :], tr_t[:w, :],
                                     mybir.ActivationFunctionType.Copy)
                nc.sync.dma_start(
                    out=bass.AP(scr, 128 * a, [[128, w], [1, 128]]),
                    in_=st_t[:w, :],
                )

        # ---- main loop over timestep blocks ----
        for bidx in range(NB):
            t0 = BLK * bidx
            nt = min(BLK, T_PAD - t0)
            # reduce tiles + scratch stores needed by this block's conv reads
            need = min(R_TILES, _ceil_div(t0 + nt + T_NO - 1, 128))
            if bidx == NB - 1:
                need = R_TILES
            while reduced < need:
                emit_reduce(reduced)
                reduced += 1
            while stored < need:
                emit_store(stored, min(need, stored + 16))
                stored = min(need, stored + 16)

            # wide Hankel tiles; K-chunk j of the window is a column slice
            vh = hkpool.tile([128, BLK + VH_W], BF, name="vh", tag="vh")
            nc.sync.dma_start(
                out=vh[:, : nt + VH_W],
                in_=bass.AP(v_h, t0, [[1, 128], [1, nt + VH_W]]),
            )
            synh = {}
            for nm, scr in (("se", sse_h), ("si", ssi_h)):
                t = hkpool.tile([128, BLK + SY_W], BF, name=f"{nm}h", tag=f"{nm}h")
                nc.sync.dma_start(
                    out=t[:, : nt + SY_W],
                    in_=bass.AP(scr, t0, [[1, 128], [1, nt + SY_W]]),
                )
                synh[nm] = t

            # layers 1..3 (hid on PSUM partitions)
            def layer(rhs_of, k_list, w_sb, bias_t, out_nm):
                outs = []
                for mc, (mo, nm_) in enumerate(m_hid):
                    ps = psmm.tile([128, BLK], FP32, name="ps", tag="ps")
                    for kc, (ko, pk) in enumerate(k_list):
                        nc.tensor.matmul(
                            ps[:nm_, :nt],
                            w_sb[kc][:pk, mo : mo + nm_],
                            rhs_of(kc, pk),
                            start=(kc == 0),
                            stop=(kc == len(k_list) - 1),
                        )
                    h_t = hpool.tile([128, BLK], BF, name=f"{out_nm}{mc}",
                                     tag=f"{out_nm}{mc}")
                    nc.scalar.activation(
                        h_t[:nm_, :nt], ps[:nm_, :nt],
                        mybir.ActivationFunctionType.Lrelu,
                        bias=bias_t[:nm_, mc : mc + 1], alpha=0.01,
                    )
                    outs.append(h_t)
                return outs

            h1 = layer(lambda kc, pk: vh[:pk, 128 * kc : 128 * kc + nt],
                       k_win, w1t_sb, bias_sb["b1"], "h1_")
            h2 = layer(lambda kc, pk: h1[kc][:pk, :nt],
                       k_hid, w2t_sb, bias_sb["b2"], "h2_")
            h3 = layer(lambda kc, pk: h2[kc][:pk, :nt],
                       k_hid, w3t_sb, bias_sb["b3"], "h3_")

            # layer 4 + conv in [sub, t] orientation; b4 via K=1 matmul
            ps4 = ps4p.tile([SUB, BLK], FP32, name="ps4", tag="ps4")
            nc.tensor.matmul(ps4[:, :nt], b4_sb[0:1, :], ones_sb[0:1, :nt],
                             start=True, stop=False)
            for kc, (ko, pk) in enumerate(k_hid):
                nc.tensor.matmul(ps4[:, :nt], w4t_sb[kc][:pk, :],
                                 h3[kc][:pk, :nt], start=False, stop=False)
            for nm, k_sb in (("se", ek_sb), ("si", ik_sb)):
                for j, (o, pk) in enumerate(k_syn):
                    last = nm == "si" and j == len(k_syn) - 1
                    nc.tensor.matmul(
                        ps4[:, :nt], k_sb[j][:pk, :],
                        synh[nm][:pk, 128 * j : 128 * j + nt],
                        start=False, stop=last,
                    )
            out_sb = opool.tile([SUB, BLK], FP32, name="out_sb", tag="out_sb")
            nc.scalar.activation(out_sb[:, :nt], ps4[:, :nt],
                                 mybir.ActivationFunctionType.Tanh)
            nc.sync.dma_start(out=out_h[:, t0 : t0 + nt], in_=out_sb[:, :nt])

    nc.compile()
    return nc


def kernel(V, S_e, S_i, W1, b1, W2, b2, W3, b3, W4, b4, W_syn, Tau_syn, Delta_syn):
    V = np.asarray(V, np.float32)
    S_e = np.ascontiguousarray(np.asarray(S_e, np.float32))
    S_i = np.ascontiguousarray(np.asarray(S_i, np.float32))
    T = V.shape[0]
    assert T % N_CORES == 0
    T_LOC = T // N_CORES
    T_PAD = _ceil_div(T_LOC, 128) * 128
    SE_ROWS = T_NO - 1 + T_LOC
    V_LEN = T_PAD + WIN - 1 + 128
    HID = W1.shape[0]
    SUB = W4.shape[0]

    # ---- tiny host-side prep (layout/dtype only + 20x200 conv kernels) ----
    W_syn = np.asarray(W_syn, np.float32)
    Tau_syn = np.asarray(Tau_syn, np.float32)
    Delta_syn = np.asarray(Delta_syn, np.float32)
    t_raw = np.arange(T_NO, dtype=np.float32)[None, :]
    t_e = np.maximum(t_raw - Delta_syn[:, 0:1], 0.0)
    t_i = np.maximum(t_raw - Delta_syn[:, 1:2], 0.0)
    tt_e = t_e / Tau_syn[:, 0:1] ** 2
    tt_i = t_i / Tau_syn[:, 1:2] ** 2
    e_kern = tt_e * np.exp(-tt_e) * W_syn[:, 0:1] ** 2
    i_kern = -(tt_i * np.exp(-tt_i)) * W_syn[:, 1:2] ** 2
    ekm = np.ascontiguousarray(e_kern[:, ::-1].T).astype(BF16)  # [T_NO, SUB]
    ikm = np.ascontiguousarray(i_kern[:, ::-1].T).astype(BF16)

    wd = {
        "w1t": np.ascontiguousarray(np.asarray(W1, np.float32).T).astype(BF16),
        "w2t": np.ascontiguousarray(np.asarray(W2, np.float32).T).astype(BF16),
        "w3t": np.ascontiguousarray(np.asarray(W3, np.float32).T).astype(BF16),
        "w4t": np.ascontiguousarray(np.asarray(W4, np.float32).T).astype(BF16),
        "b1": np.asarray(b1, np.float32),
        "b2": np.asarray(b2, np.float32),
        "b3": np.asarray(b3, np.float32),
        "b4": np.asarray(b4, np.float32).astype(BF16),
        "ekm": ekm,
        "ikm": ikm,
    }

    vg = np.zeros(T_NO - 1 + T + WIN + 128 + T_PAD - T_LOC, np.float32)
    vg[T_NO - 1 : T_NO - 1 + T] = V
    vg = vg.astype(BF16)

    halo = T_NO - 1
    ez = np.zeros((halo, S_e.shape[1]), np.float32)
    iz = np.zeros((halo, S_i.shape[1]), np.float32)
    in_maps = []
    for m in range(N_CORES):
        r0 = m * T_LOC
        if m == 0:
            se_m = np.concatenate([ez, S_e[:T_LOC]], 0)
            si_m = np.concatenate([iz, S_i[:T_LOC]], 0)
        else:
            se_m = S_e[r0 - halo : r0 + T_LOC]
            si_m = S_i[r0 - halo : r0 + T_LOC]
        in_maps.append(
            {"se": se_m, "si": si_m, "v": vg[r0 : r0 + V_LEN], **wd}
        )

    nc = _build(T_PAD, SE_ROWS, S_e.shape[1], S_i.shape[1], HID, SUB)
    trace = os.environ.get("CC_TRACE") == "1"
    res = run_bass_kernel_spmd(nc, in_maps, list(range(N_CORES)), trace=trace)
    LAST["exec_time_ns"] = res.exec_time_ns
    LAST["results"] = res
    out = np.concatenate(
        [res.results[m]["out"][:, :T_LOC].T for m in range(N_CORES)], 0
    )
    return np.ascontiguousarray(out.astype(np.float32))


# revision 6
# speedup vs baseline: 2.0174x; 1.1010x over previous
"""Trainium2 Bass kernel for nn_Encoder (sliding-window MLP + synaptic conv).

Computation (per timestep t of T_data):
  syn_e[t] = sum(S_e[t, :]);  syn_i[t] = sum(S_i[t, :])
  syn_out[t, s] = sum_k e_kern[s, k] * syn_e[t-k] + i_kern[s, k] * syn_i[t-k]
  Vw[t, :] = V[t-199 : t+200]   (zero padded)
  h = lrelu(Vw @ W1.T + b1); h = lrelu(h @ W2.T + b2); h = lrelu(h @ W3.T + b3)
  out[t, :] = tanh(h @ W4.T + b4 + syn_out[t, :])

Strategy: data-parallel over T across 8 NeuronCores, each core gets its
T/8 slice plus a 199-row halo of S_e/S_i and a 398-elem halo of V (host
zero-pads the edges).  On each core:
  - S_e/S_i row-sums via VectorE free-axis reduce (fp32), PE-transposed and
    cast to bf16, stored contiguously to a DRAM scratch vector.
  - All matmuls in bf16 (fp32 PSUM accumulate).  Sliding windows of V and
    syn_e/syn_i are materialized as matmul operands directly by overlapping
    ("Hankel") DMA access patterns from DRAM: partition step 1, free step 1.
    One wide Hankel tile per block serves all K-chunks as column slices.
  - Layers 1-3 keep hid on PSUM partitions; layer 4 + conv keep the
    [sub, t] orientation (weights stationary) so the output store is a
    contiguous [sub, nt] tile; the host transposes the gathered output.
    b4 is added via a K=1 matmul (b4 stationary, ones streaming).
  - lrelu/tanh run on ScalarE (Lrelu alpha=0.01, Tanh); both live in the
    same ACT LUT table set so only one table load is emitted.
"""

import os
from contextlib import ExitStack

import ml_dtypes
import numpy as np

import concourse.bass as bass
import concourse.mybir as mybir
import concourse.tile as tile
from concourse import bacc
from concourse.bass_utils import run_bass_kernel_spmd
from concourse.masks import make_identity

BF16 = ml_dtypes.bfloat16
FP32 = mybir.dt.float32
BF = mybir.dt.bfloat16

T_NO = 200
WIN = 2 * T_NO - 1  # 399
N_CORES = 8
BLK = 512  # timesteps per block (one PSUM bank of fp32)

LAST = {}  # exec_time_ns / trace info from the most recent run (for test harness)


def _ceil_div(a, b):
    return -(-a // b)


def _chunks(total, step=128):
    out = []
    o = 0
    while o < total:
        out.append((o, min(step, total - o)))
        o += step
    return out


def _build(T_PAD, SE_ROWS, E_COLS, I_COLS, HID, SUB):
    """Build the per-core Bass program (identical on all 8 cores)."""
    R_TILES = _ceil_div(SE_ROWS, 128)
    # +128 margin: the wide hankel DMA loads a full [128, nt+W] rectangle
    # whose unused corner reads past the logical end
    SCR_LEN = R_TILES * 128 + 128
    V_LEN = T_PAD + WIN - 1 + 128
    NB = _ceil_div(T_PAD, BLK)

    m_hid = _chunks(HID)  # M chunks of hid (PSUM partitions L1-3)
    k_win = _chunks(WIN)  # K chunks of the V window
    k_hid = _chunks(HID)  # K chunks of hid (L2-4 contraction)
    k_syn = _chunks(T_NO)  # K chunks of the conv kernel length
    VH_W = 128 * (len(k_win) - 1)  # extra hankel cols so K-chunks are slices
    SY_W = 128 * (len(k_syn) - 1)

    nc = bacc.Bacc(
        "TRN2", target_bir_lowering=False, debug=False, num_devices=N_CORES
    )

    se_h = nc.dram_tensor("se", [SE_ROWS, E_COLS], FP32, kind="ExternalInput")
    si_h = nc.dram_tensor("si", [SE_ROWS, I_COLS], FP32, kind="ExternalInput")
    v_h = nc.dram_tensor("v", [V_LEN], BF, kind="ExternalInput")
    w1t_h = nc.dram_tensor("w1t", [WIN, HID], BF, kind="ExternalInput")
    w2t_h = nc.dram_tensor("w2t", [HID, HID], BF, kind="ExternalInput")
    w3t_h = nc.dram_tensor("w3t", [HID, HID], BF, kind="ExternalInput")
    w4t_h = nc.dram_tensor("w4t", [HID, SUB], BF, kind="ExternalInput")
    b1_h = nc.dram_tensor("b1", [HID], FP32, kind="ExternalInput")
    b2_h = nc.dram_tensor("b2", [HID], FP32, kind="ExternalInput")
    b3_h = nc.dram_tensor("b3", [HID], FP32, kind="ExternalInput")
    b4_h = nc.dram_tensor("b4", [SUB], BF, kind="ExternalInput")
    ekm_h = nc.dram_tensor("ekm", [T_NO, SUB], BF, kind="ExternalInput")
    ikm_h = nc.dram_tensor("ikm", [T_NO, SUB], BF, kind="ExternalInput")
    out_h = nc.dram_tensor("out", [SUB, T_PAD], FP32, kind="ExternalOutput")

    sse_h = nc.dram_tensor("sse_scratch", [SCR_LEN], BF)
    ssi_h = nc.dram_tensor("ssi_scratch", [SCR_LEN], BF)

    with tile.TileContext(nc) as tc, ExitStack() as ctx:
        cpool = ctx.enter_context(tc.tile_pool(name="consts", bufs=1))
        sepool = ctx.enter_context(tc.tile_pool(name="sein", bufs=5))
        accpool = ctx.enter_context(tc.tile_pool(name="acc", bufs=1))
        stpool = ctx.enter_context(tc.tile_pool(name="store", bufs=2))
        hkpool = ctx.enter_context(tc.tile_pool(name="hankel", bufs=2))
        hpool = ctx.enter_context(tc.tile_pool(name="acts", bufs=2))
        opool = ctx.enter_context(tc.tile_pool(name="outs", bufs=3))
        psmm = ctx.enter_context(tc.tile_pool(name="psmm", bufs=5, space="PSUM"))
        ps4p = ctx.enter_context(tc.tile_pool(name="ps4p", bufs=2, space="PSUM"))
        ptrp = ctx.enter_context(tc.tile_pool(name="ptrp", bufs=1, space="PSUM"))

        # ---- constants to SBUF ----
        def load_rows(dram, rows, cols, dt, nm):
            tiles = []
            for j, (o, p) in enumerate(rows):
                t = cpool.tile([128, cols], dt, name=f"{nm}{j}", tag=f"{nm}{j}")
                nc.sync.dma_start(out=t[:p, :], in_=dram[o : o + p, :])
                tiles.append(t)
            return tiles

        w1t_sb = load_rows(w1t_h, k_win, HID, BF, "w1t")
        w2t_sb = load_rows(w2t_h, k_hid, HID, BF, "w2t")
        w3t_sb = load_rows(w3t_h, k_hid, HID, BF, "w3t")
        w4t_sb = load_rows(w4t_h, k_hid, SUB, BF, "w4t")
        ek_sb = load_rows(ekm_h, k_syn, SUB, BF, "ek")
        ik_sb = load_rows(ikm_h, k_syn, SUB, BF, "ik")

        bias_sb = {}
        for nm, h in (("b1", b1_h), ("b2", b2_h), ("b3", b3_h)):
            t = cpool.tile([128, len(m_hid)], FP32, name=nm, tag=nm)
            for c, (o, p) in enumerate(m_hid):
                nc.sync.dma_start(out=t[:p, c], in_=h[o : o + p])
            bias_sb[nm] = t

        b4_sb = cpool.tile([1, SUB], BF, name="b4sb", tag="b4sb")
        nc.sync.dma_start(out=b4_sb[0:1, :], in_=bass.AP(b4_h, 0, [[0, 1], [1, SUB]]))
        ones_sb = cpool.tile([1, BLK], BF, name="ones", tag="ones")
        nc.vector.memset(ones_sb[0:1, :], 1.0)
        ident = cpool.tile([128, 128], FP32, name="ident", tag="ident")
        make_identity(nc, ident[:, :])

        # ---- reduction accumulators ----
        se_acc = accpool.tile([128, R_TILES], FP32, name="se_acc", tag="se_acc")
        si_acc = accpool.tile([128, R_TILES], FP32, name="si_acc", tag="si_acc")
        if SE_ROWS % 128 != 0:
            # rows past SE_ROWS in the last reduce tile are never written by
            # the reduce; zero them so the scratch tail holds no garbage
            nc.vector.memset(se_acc[:, R_TILES - 1 : R_TILES], 0.0)
            nc.vector.memset(si_acc[:, R_TILES - 1 : R_TILES], 0.0)

        reduced = 0  # reduce tiles emitted so far
        stored = 0  # scratch columns stored so far

        def emit_reduce(i):
            r0 = 128 * i
            nr = min(128, SE_ROWS - r0)
            se_t = sepool.tile([128, E_COLS], FP32, name="se_t", tag="se_t")
            nc.gpsimd.dma_start(out=se_t[:nr, :], in_=se_h[r0 : r0 + nr, :])
            nc.vector.reduce_sum(se_acc[:nr, i : i + 1], se_t[:nr, :],
                                 axis=mybir.AxisListType.X)
            si_t = sepool.tile([128, I_COLS], FP32, name="si_t", tag="si_t")
            nc.gpsimd.dma_start(out=si_t[:nr, :], in_=si_h[r0 : r0 + nr, :])
            nc.vector.reduce_sum(si_acc[:nr, i : i + 1], si_t[:nr, :],
                                 axis=mybir.AxisListType.X)

        def emit_store(a, b):
            # PE-transpose fp32 accumulator cols [a,b) to [w,128], cast to
            # bf16 on ScalarE, store contiguously to the scratch vector
            w = b - a
            for nm, acc, scr in (("se", se_acc, sse_h), ("si", si_acc, ssi_h)):
                tr_t = ptrp.tile([16, 128], FP32, name=f"{nm}tr", tag="tr")
                nc.tensor.transpose(tr_t[:w, :], acc[:, a:b], ident[:, :])
                st_t = stpool.tile([16, 128], BF, name=f"{nm}st", tag=f"{nm}st")
                nc.scalar.activation(st_t[:w, :], tr_t[:w, :],
                                     mybir.ActivationFunctionType.Copy)
                nc.sync.dma_start(
                    out=bass.AP(scr, 128 * a, [[128, w], [1, 128]]),
                    in_=st_t[:w, :],
                )

        # ---- main loop over timestep block pairs ----
        # Blocks are processed in pairs, interleaved at layer granularity:
        # L1(a) L1(b) L2(a) L2(b) ... so the PE never waits for the ScalarE
        # PSUM evacuation of the previous layer (it runs under the other
        # block's matmuls) and stays HAM-warm.  One wide Hankel tile per
        # pair serves both blocks and all K-chunks as column slices.
        pairs = [tuple(b for b in (i, i + 1) if b < NB) for i in range(0, NB, 2)]
        for pair in pairs:
            t0p = BLK * pair[0]
            blks = []
            off = 0
            for b in pair:
                nt = min(BLK, T_PAD - BLK * b)
                blks.append((BLK * b, nt, off))
                off += nt
            tot = off

            last_t0, last_nt, _ = blks[-1]
            need = min(R_TILES, _ceil_div(last_t0 + last_nt + T_NO - 1, 128))
            if pair is pairs[-1]:
                need = R_TILES
            while reduced < need:
                emit_reduce(reduced)
                reduced += 1
            while stored < need:
                emit_store(stored, min(need, stored + 16))
                stored = min(need, stored + 16)

            vh = hkpool.tile([128, 2 * BLK + VH_W], BF, name="vh", tag="vh")
            nc.sync.dma_start(
                out=vh[:, : tot + VH_W],
                in_=bass.AP(v_h, t0p, [[1, 128], [1, tot + VH_W]]),
            )
            synh = {}
            for nm, scr in (("se", sse_h), ("si", ssi_h)):
                t = hkpool.tile([128, 2 * BLK + SY_W], BF, name=f"{nm}h",
                                tag=f"{nm}h")
                nc.sync.dma_start(
                    out=t[:, : tot + SY_W],
                    in_=bass.AP(scr, t0p, [[1, 128], [1, tot + SY_W]]),
                )
                synh[nm] = t

            # layers 1..3 (hid on PSUM partitions), block-pair interleaved
            h_prev = {}  # (layer, block-slot) -> list of 4 sbuf tiles
            for lidx, (k_list, w_sb, bias_t) in enumerate((
                (k_win, w1t_sb, bias_sb["b1"]),
                (k_hid, w2t_sb, bias_sb["b2"]),
                (k_hid, w3t_sb, bias_sb["b3"]),
            )):
                for si_, (bt0, nt, coff) in enumerate(blks):
                    outs = []
                    for mc, (mo, nm_) in enumerate(m_hid):
                        ps = psmm.tile([128, BLK], FP32, name="ps", tag="ps")
                        for kc, (ko, pk) in enumerate(k_list):
                            if lidx == 0:
                                rhs = vh[:pk, coff + 128 * kc : coff + 128 * kc + nt]
                            else:
                                rhs = h_prev[si_][kc][:pk, :nt]
                            nc.tensor.matmul(
                                ps[:nm_, :nt],
                                w_sb[kc][:pk, mo : mo + nm_],
                                rhs,
                                start=(kc == 0),
                                stop=(kc == len(k_list) - 1),
                            )
                        h_t = hpool.tile([128, BLK], BF, name=f"h{lidx}_{mc}",
                                         tag=f"h{lidx}_{mc}")
                        nc.scalar.activation(
                            h_t[:nm_, :nt], ps[:nm_, :nt],
                            mybir.ActivationFunctionType.Lrelu,
                            bias=bias_t[:nm_, mc : mc + 1], alpha=0.01,
                        )
                        outs.append(h_t)
                    h_prev[si_] = outs

            # layer 4 + conv in [sub, t] orientation; b4 via K=1 matmul
            for si_, (bt0, nt, coff) in enumerate(blks):
                h3 = h_prev[si_]
                ps4 = ps4p.tile([SUB, BLK], FP32, name="ps4", tag="ps4")
                nc.tensor.matmul(ps4[:, :nt], b4_sb[0:1, :], ones_sb[0:1, :nt],
                                 start=True, stop=False)
                for kc, (ko, pk) in enumerate(k_hid):
                    nc.tensor.matmul(ps4[:, :nt], w4t_sb[kc][:pk, :],
                                     h3[kc][:pk, :nt], start=False, stop=False)
                for nm, k_sb in (("se", ek_sb), ("si", ik_sb)):
                    for j, (o, pk) in enumerate(k_syn):
                        last = nm == "si" and j == len(k_syn) - 1
                        nc.tensor.matmul(
                            ps4[:, :nt], k_sb[j][:pk, :],
                            synh[nm][:pk, coff + 128 * j : coff + 128 * j + nt],
                            start=False, stop=last,
                        )
                out_sb = opool.tile([SUB, BLK], FP32, name="out_sb", tag="out_sb")
                nc.scalar.activation(out_sb[:, :nt], ps4[:, :nt],
                                     mybir.ActivationFunctionType.Tanh)
                nc.sync.dma_start(out=out_h[:, bt0 : bt0 + nt], in_=out_sb[:, :nt])

    nc.compile()
    return nc


def kernel(V, S_e, S_i, W1, b1, W2, b2, W3, b3, W4, b4, W_syn, Tau_syn, Delta_syn):
    V = np.asarray(V, np.float32)
    S_e = np.ascontiguousarray(np.asarray(S_e, np.float32))
    S_i = np.ascontiguousarray(np.asarray(S_i, np.float32))
    T = V.shape[0]
    assert T % N_CORES == 0
    T_LOC = T // N_CORES
    T_PAD = _ceil_div(T_LOC, 128) * 128
    SE_ROWS = T_NO - 1 + T_LOC
    V_LEN = T_PAD + WIN - 1 + 128
    HID = W1.shape[0]
    SUB = W4.shape[0]

    # ---- tiny host-side prep (layout/dtype only + 20x200 conv kernels) ----
    W_syn = np.asarray(W_syn, np.float32)
    Tau_syn = np.asarray(Tau_syn, np.float32)
    Delta_syn = np.asarray(Delta_syn, np.float32)
    t_raw = np.arange(T_NO, dtype=np.float32)[None, :]
    t_e = np.maximum(t_raw - Delta_syn[:, 0:1], 0.0)
    t_i = np.maximum(t_raw - Delta_syn[:, 1:2], 0.0)
    tt_e = t_e / Tau_syn[:, 0:1] ** 2
    tt_i = t_i / Tau_syn[:, 1:2] ** 2
    e_kern = tt_e * np.exp(-tt_e) * W_syn[:, 0:1] ** 2
    i_kern = -(tt_i * np.exp(-tt_i)) * W_syn[:, 1:2] ** 2
    ekm = np.ascontiguousarray(e_kern[:, ::-1].T).astype(BF16)  # [T_NO, SUB]
    ikm = np.ascontiguousarray(i_kern[:, ::-1].T).astype(BF16)

    wd = {
        "w1t": np.ascontiguousarray(np.asarray(W1, np.float32).T).astype(BF16),
        "w2t": np.ascontiguousarray(np.asarray(W2, np.float32).T).astype(BF16),
        "w3t": np.ascontiguousarray(np.asarray(W3, np.float32).T).astype(BF16),
        "w4t": np.ascontiguousarray(np.asarray(W4, np.float32).T).astype(BF16),
        "b1": np.asarray(b1, np.float32),
        "b2": np.asarray(b2, np.float32),
        "b3": np.asarray(b3, np.float32),
        "b4": np.asarray(b4, np.float32).astype(BF16),
        "ekm": ekm,
        "ikm": ikm,
    }

    vg = np.zeros(T_NO - 1 + T + WIN + 128 + T_PAD - T_LOC, np.float32)
    vg[T_NO - 1 : T_NO - 1 + T] = V
    vg = vg.astype(BF16)

    halo = T_NO - 1
    ez = np.zeros((halo, S_e.shape[1]), np.float32)
    iz = np.zeros((halo, S_i.shape[1]), np.float32)
    in_maps = []
    for m in range(N_CORES):
        r0 = m * T_LOC
        if m == 0:
            se_m = np.concatenate([ez, S_e[:T_LOC]], 0)
            si_m = np.concatenate([iz, S_i[:T_LOC]], 0)
        else:
            se_m = S_e[r0 - halo : r0 + T_LOC]
            si_m = S_i[r0 - halo : r0 + T_LOC]
        in_maps.append(
            {"se": se_m, "si": si_m, "v": vg[r0 : r0 + V_LEN], **wd}
        )

    nc = _build(T_PAD, SE_ROWS, S_e.shape[1], S_i.shape[1], HID, SUB)
    trace = os.environ.get("CC_TRACE") == "1"
    res = run_bass_kernel_spmd(nc, in_maps, list(range(N_CORES)), trace=trace)
    LAST["exec_time_ns"] = res.exec_time_ns
    LAST["results"] = res
    out = np.concatenate(
        [res.results[m]["out"][:, :T_LOC].T for m in range(N_CORES)], 0
    )
    return np.ascontiguousarray(out.astype(np.float32))


# revision 7
# speedup vs baseline: 2.1239x; 1.0528x over previous
"""Trainium2 Bass kernel for nn_Encoder (sliding-window MLP + synaptic conv).

Computation (per timestep t of T_data):
  syn_e[t] = sum(S_e[t, :]);  syn_i[t] = sum(S_i[t, :])
  syn_out[t, s] = sum_k e_kern[s, k] * syn_e[t-k] + i_kern[s, k] * syn_i[t-k]
  Vw[t, :] = V[t-199 : t+200]   (zero padded)
  h = lrelu(Vw @ W1.T + b1); h = lrelu(h @ W2.T + b2); h = lrelu(h @ W3.T + b3)
  out[t, :] = tanh(h @ W4.T + b4 + syn_out[t, :])

Strategy: data-parallel over T across 8 NeuronCores, each core gets its
T/8 slice plus a 199-row halo of S_e/S_i and a 398-elem halo of V (host
zero-pads the edges).  On each core:
  - S_e/S_i row-sums via VectorE free-axis reduce (fp32), PE-transposed and
    cast to bf16, stored contiguously to a DRAM scratch vector.
  - All matmuls in bf16 (fp32 PSUM accumulate).  Sliding windows of V and
    syn_e/syn_i are materialized as matmul operands directly by overlapping
    ("Hankel") DMA access patterns from DRAM: partition step 1, free step 1.
    One wide Hankel tile per block serves all K-chunks as column slices.
  - Layers 1-3 keep hid on PSUM partitions; layer 4 + conv keep the
    [sub, t] orientation (weights stationary) so the output store is a
    contiguous [sub, nt] tile; the host transposes the gathered output.
    b4 is added via a K=1 matmul (b4 stationary, ones streaming).
  - lrelu/tanh run on ScalarE (Lrelu alpha=0.01, Tanh); both live in the
    same ACT LUT table set so only one table load is emitted.
"""

import os
from contextlib import ExitStack

import ml_dtypes
import numpy as np

import concourse.bass as bass
import concourse.mybir as mybir
import concourse.tile as tile
from concourse import bacc
from concourse.bass_utils import run_bass_kernel_spmd
from concourse.masks import make_identity

BF16 = ml_dtypes.bfloat16
FP32 = mybir.dt.float32
BF = mybir.dt.bfloat16

T_NO = 200
WIN = 2 * T_NO - 1  # 399
N_CORES = 8
BLK = 512  # timesteps per block (one PSUM bank of fp32)

LAST = {}  # exec_time_ns / trace info from the most recent run (for test harness)


def _ceil_div(a, b):
    return -(-a // b)


def _chunks(total, step=128):
    out = []
    o = 0
    while o < total:
        out.append((o, min(step, total - o)))
        o += step
    return out


def _build(T_PAD, SE_ROWS, E_COLS, I_COLS, HID, SUB):
    """Build the per-core Bass program (identical on all 8 cores)."""
    R_TILES = _ceil_div(SE_ROWS, 128)
    # +128 margin: the wide hankel DMA loads a full [128, nt+W] rectangle
    # whose unused corner reads past the logical end
    SCR_LEN = R_TILES * 128 + 128
    V_LEN = T_PAD + WIN - 1 + 128
    NB = _ceil_div(T_PAD, BLK)

    m_hid = _chunks(HID)  # M chunks of hid (PSUM partitions L1-3)
    k_win = _chunks(WIN)  # K chunks of the V window
    k_hid = _chunks(HID)  # K chunks of hid (L2-4 contraction)
    k_syn = _chunks(T_NO)  # K chunks of the conv kernel length
    VH_W = 128 * (len(k_win) - 1)  # extra hankel cols so K-chunks are slices
    SY_W = 128 * (len(k_syn) - 1)

    nc = bacc.Bacc(
        "TRN2", target_bir_lowering=False, debug=False, num_devices=N_CORES
    )

    se_h = nc.dram_tensor("se", [SE_ROWS, E_COLS], FP32, kind="ExternalInput")
    si_h = nc.dram_tensor("si", [SE_ROWS, I_COLS], FP32, kind="ExternalInput")
    v_h = nc.dram_tensor("v", [V_LEN], BF, kind="ExternalInput")
    w1t_h = nc.dram_tensor("w1t", [WIN, HID], BF, kind="ExternalInput")
    w2t_h = nc.dram_tensor("w2t", [HID, HID], BF, kind="ExternalInput")
    w3t_h = nc.dram_tensor("w3t", [HID, HID], BF, kind="ExternalInput")
    w4t_h = nc.dram_tensor("w4t", [HID, SUB], BF, kind="ExternalInput")
    b1_h = nc.dram_tensor("b1", [HID], FP32, kind="ExternalInput")
    b2_h = nc.dram_tensor("b2", [HID], FP32, kind="ExternalInput")
    b3_h = nc.dram_tensor("b3", [HID], FP32, kind="ExternalInput")
    b4_h = nc.dram_tensor("b4", [SUB], BF, kind="ExternalInput")
    ekm_h = nc.dram_tensor("ekm", [T_NO, SUB], BF, kind="ExternalInput")
    ikm_h = nc.dram_tensor("ikm", [T_NO, SUB], BF, kind="ExternalInput")
    out_h = nc.dram_tensor("out", [SUB, T_PAD], FP32, kind="ExternalOutput")

    sse_h = nc.dram_tensor("sse_scratch", [SCR_LEN], BF)
    ssi_h = nc.dram_tensor("ssi_scratch", [SCR_LEN], BF)

    with tile.TileContext(nc) as tc, ExitStack() as ctx:
        cpool = ctx.enter_context(tc.tile_pool(name="consts", bufs=1))
        sepool = ctx.enter_context(tc.tile_pool(name="sein", bufs=8))
        accpool = ctx.enter_context(tc.tile_pool(name="acc", bufs=1))
        stpool = ctx.enter_context(tc.tile_pool(name="store", bufs=2))
        hkpool = ctx.enter_context(tc.tile_pool(name="hankel", bufs=2))
        hpool = ctx.enter_context(tc.tile_pool(name="acts", bufs=2))
        opool = ctx.enter_context(tc.tile_pool(name="outs", bufs=3))
        psmm = ctx.enter_context(tc.tile_pool(name="psmm", bufs=5, space="PSUM"))
        ps4p = ctx.enter_context(tc.tile_pool(name="ps4p", bufs=2, space="PSUM"))
        ptrp = ctx.enter_context(tc.tile_pool(name="ptrp", bufs=1, space="PSUM"))

        # ---- constants to SBUF ----
        def load_rows(dram, rows, cols, dt, nm):
            tiles = []
            for j, (o, p) in enumerate(rows):
                t = cpool.tile([128, cols], dt, name=f"{nm}{j}", tag=f"{nm}{j}")
                nc.sync.dma_start(out=t[:p, :], in_=dram[o : o + p, :])
                tiles.append(t)
            return tiles

        w1t_sb = load_rows(w1t_h, k_win, HID, BF, "w1t")
        w2t_sb = load_rows(w2t_h, k_hid, HID, BF, "w2t")
        w3t_sb = load_rows(w3t_h, k_hid, HID, BF, "w3t")
        w4t_sb = load_rows(w4t_h, k_hid, SUB, BF, "w4t")
        ek_sb = load_rows(ekm_h, k_syn, SUB, BF, "ek")
        ik_sb = load_rows(ikm_h, k_syn, SUB, BF, "ik")

        bias_sb = {}
        for nm, h in (("b1", b1_h), ("b2", b2_h), ("b3", b3_h)):
            t = cpool.tile([128, len(m_hid)], FP32, name=nm, tag=nm)
            for c, (o, p) in enumerate(m_hid):
                nc.sync.dma_start(out=t[:p, c], in_=h[o : o + p])
            bias_sb[nm] = t

        b4_sb = cpool.tile([1, SUB], BF, name="b4sb", tag="b4sb")
        nc.sync.dma_start(out=b4_sb[0:1, :], in_=bass.AP(b4_h, 0, [[0, 1], [1, SUB]]))
        ones_sb = cpool.tile([1, BLK], BF, name="ones", tag="ones")
        nc.vector.memset(ones_sb[0:1, :], 1.0)
        ident = cpool.tile([128, 128], FP32, name="ident", tag="ident")
        make_identity(nc, ident[:, :])

        # ---- reduction accumulators ----
        se_acc = accpool.tile([128, R_TILES], FP32, name="se_acc", tag="se_acc")
        si_acc = accpool.tile([128, R_TILES], FP32, name="si_acc", tag="si_acc")
        if SE_ROWS % 128 != 0:
            # rows past SE_ROWS in the last reduce tile are never written by
            # the reduce; zero them so the scratch tail holds no garbage
            nc.vector.memset(se_acc[:, R_TILES - 1 : R_TILES], 0.0)
            nc.vector.memset(si_acc[:, R_TILES - 1 : R_TILES], 0.0)

        reduced = 0  # reduce tiles emitted so far
        stored = 0  # scratch columns stored so far

        def emit_reduce(i):
            r0 = 128 * i
            nr = min(128, SE_ROWS - r0)
            se_t = sepool.tile([128, E_COLS], FP32, name="se_t", tag="se_t")
            nc.gpsimd.dma_start(out=se_t[:nr, :], in_=se_h[r0 : r0 + nr, :])
            nc.vector.reduce_sum(se_acc[:nr, i : i + 1], se_t[:nr, :],
                                 axis=mybir.AxisListType.X)
            si_t = sepool.tile([128, I_COLS], FP32, name="si_t", tag="si_t")
            nc.gpsimd.dma_start(out=si_t[:nr, :], in_=si_h[r0 : r0 + nr, :])
            nc.vector.reduce_sum(si_acc[:nr, i : i + 1], si_t[:nr, :],
                                 axis=mybir.AxisListType.X)

        def emit_store(a, b):
            # PE-transpose fp32 accumulator cols [a,b) to [w,128], cast to
            # bf16 on ScalarE, store contiguously to the scratch vector
            w = b - a
            for nm, acc, scr in (("se", se_acc, sse_h), ("si", si_acc, ssi_h)):
                tr_t = ptrp.tile([16, 128], FP32, name=f"{nm}tr", tag="tr")
                nc.tensor.transpose(tr_t[:w, :], acc[:, a:b], ident[:, :])
                st_t = stpool.tile([16, 128], BF, name=f"{nm}st", tag=f"{nm}st")
                nc.scalar.activation(st_t[:w, :], tr_t[:w, :],
                                     mybir.ActivationFunctionType.Copy)
                nc.scalar.dma_start(
                    out=bass.AP(scr, 128 * a, [[128, w], [1, 128]]),
                    in_=st_t[:w, :],
                )

        # ---- main loop over timestep block pairs ----
        # Blocks are processed in pairs, interleaved at layer granularity:
        # L1(a) L1(b) L2(a) L2(b) ... so the PE never waits for the ScalarE
        # PSUM evacuation of the previous layer (it runs under the other
        # block's matmuls) and stays HAM-warm.  One wide Hankel tile per
        # pair serves both blocks and all K-chunks as column slices.
        pairs = [tuple(b for b in (i, i + 1) if b < NB) for i in range(0, NB, 2)]
        for pair in pairs:
            t0p = BLK * pair[0]
            blks = []
            off = 0
            for b in pair:
                nt = min(BLK, T_PAD - BLK * b)
                blks.append((BLK * b, nt, off))
                off += nt
            tot = off

            last_t0, last_nt, _ = blks[-1]
            need = min(R_TILES, _ceil_div(last_t0 + last_nt + T_NO - 1, 128))
            if pair is pairs[-1]:
                need = R_TILES
            while reduced < need:
                emit_reduce(reduced)
                reduced += 1
            vh = hkpool.tile([128, 2 * BLK + VH_W], BF, name="vh", tag="vh")
            nc.sync.dma_start(
                out=vh[:, : tot + VH_W],
                in_=bass.AP(v_h, t0p, [[1, 128], [1, tot + VH_W]]),
            )

            # layers 1..3 (hid on PSUM partitions), block-pair interleaved
            h_prev = {}  # (layer, block-slot) -> list of 4 sbuf tiles
            for lidx, (k_list, w_sb, bias_t) in enumerate((
                (k_win, w1t_sb, bias_sb["b1"]),
                (k_hid, w2t_sb, bias_sb["b2"]),
                (k_hid, w3t_sb, bias_sb["b3"]),
            )):
                for si_, (bt0, nt, coff) in enumerate(blks):
                    outs = []
                    for mc, (mo, nm_) in enumerate(m_hid):
                        ps = psmm.tile([128, BLK], FP32, name="ps", tag="ps")
                        for kc, (ko, pk) in enumerate(k_list):
                            if lidx == 0:
                                rhs = vh[:pk, coff + 128 * kc : coff + 128 * kc + nt]
                            else:
                                rhs = h_prev[si_][kc][:pk, :nt]
                            nc.tensor.matmul(
                                ps[:nm_, :nt],
                                w_sb[kc][:pk, mo : mo + nm_],
                                rhs,
                                start=(kc == 0),
                                stop=(kc == len(k_list) - 1),
                            )
                        h_t = hpool.tile([128, BLK], BF, name=f"h{lidx}_{mc}",
                                         tag=f"h{lidx}_{mc}")
                        nc.scalar.activation(
                            h_t[:nm_, :nt], ps[:nm_, :nt],
                            mybir.ActivationFunctionType.Lrelu,
                            bias=bias_t[:nm_, mc : mc + 1], alpha=0.01,
                        )
                        outs.append(h_t)
                    h_prev[si_] = outs

            # scratch stores (PE transpose) go here: their reduce inputs
            # completed during L1-3; the conv reads below depend on them
            while stored < need:
                emit_store(stored, min(need, stored + 16))
                stored = min(need, stored + 16)
            synh = {}
            for nm, scr in (("se", sse_h), ("si", ssi_h)):
                t = hkpool.tile([128, 2 * BLK + SY_W], BF, name=f"{nm}h",
                                tag=f"{nm}h")
                nc.sync.dma_start(
                    out=t[:, : tot + SY_W],
                    in_=bass.AP(scr, t0p, [[1, 128], [1, tot + SY_W]]),
                )
                synh[nm] = t

            # layer 4 + conv in [sub, t] orientation; b4 via K=1 matmul
            for si_, (bt0, nt, coff) in enumerate(blks):
                h3 = h_prev[si_]
                ps4 = ps4p.tile([SUB, BLK], FP32, name="ps4", tag="ps4")
                nc.tensor.matmul(ps4[:, :nt], b4_sb[0:1, :], ones_sb[0:1, :nt],
                                 start=True, stop=False)
                for kc, (ko, pk) in enumerate(k_hid):
                    nc.tensor.matmul(ps4[:, :nt], w4t_sb[kc][:pk, :],
                                     h3[kc][:pk, :nt], start=False, stop=False)
                for nm, k_sb in (("se", ek_sb), ("si", ik_sb)):
                    for j, (o, pk) in enumerate(k_syn):
                        last = nm == "si" and j == len(k_syn) - 1
                        nc.tensor.matmul(
                            ps4[:, :nt], k_sb[j][:pk, :],
                            synh[nm][:pk, coff + 128 * j : coff + 128 * j + nt],
                            start=False, stop=last,
                        )
                out_sb = opool.tile([SUB, BLK], FP32, name="out_sb", tag="out_sb")
                nc.scalar.activation(out_sb[:, :nt], ps4[:, :nt],
                                     mybir.ActivationFunctionType.Tanh)
                nc.sync.dma_start(out=out_h[:, bt0 : bt0 + nt], in_=out_sb[:, :nt])

    nc.compile()
    return nc


def kernel(V, S_e, S_i, W1, b1, W2, b2, W3, b3, W4, b4, W_syn, Tau_syn, Delta_syn):
    V = np.asarray(V, np.float32)
    S_e = np.ascontiguousarray(np.asarray(S_e, np.float32))
    S_i = np.ascontiguousarray(np.asarray(S_i, np.float32))
    T = V.shape[0]
    assert T % N_CORES == 0
    T_LOC = T // N_CORES
    T_PAD = _ceil_div(T_LOC, 128) * 128
    SE_ROWS = T_NO - 1 + T_LOC
    V_LEN = T_PAD + WIN - 1 + 128
    HID = W1.shape[0]
    SUB = W4.shape[0]

    # ---- tiny host-side prep (layout/dtype only + 20x200 conv kernels) ----
    W_syn = np.asarray(W_syn, np.float32)
    Tau_syn = np.asarray(Tau_syn, np.float32)
    Delta_syn = np.asarray(Delta_syn, np.float32)
    t_raw = np.arange(T_NO, dtype=np.float32)[None, :]
    t_e = np.maximum(t_raw - Delta_syn[:, 0:1], 0.0)
    t_i = np.maximum(t_raw - Delta_syn[:, 1:2], 0.0)
    tt_e = t_e / Tau_syn[:, 0:1] ** 2
    tt_i = t_i / Tau_syn[:, 1:2] ** 2
    e_kern = tt_e * np.exp(-tt_e) * W_syn[:, 0:1] ** 2
    i_kern = -(tt_i * np.exp(-tt_i)) * W_syn[:, 1:2] ** 2
    ekm = np.ascontiguousarray(e_kern[:, ::-1].T).astype(BF16)  # [T_NO, SUB]
    ikm = np.ascontiguousarray(i_kern[:, ::-1].T).astype(BF16)

    wd = {
        "w1t": np.ascontiguousarray(np.asarray(W1, np.float32).T).astype(BF16),
        "w2t": np.ascontiguousarray(np.asarray(W2, np.float32).T).astype(BF16),
        "w3t": np.ascontiguousarray(np.asarray(W3, np.float32).T).astype(BF16),
        "w4t": np.ascontiguousarray(np.asarray(W4, np.float32).T).astype(BF16),
        "b1": np.asarray(b1, np.float32),
        "b2": np.asarray(b2, np.float32),
        "b3": np.asarray(b3, np.float32),
        "b4": np.asarray(b4, np.float32).astype(BF16),
        "ekm": ekm,
        "ikm": ikm,
    }

    vg = np.zeros(T_NO - 1 + T + WIN + 128 + T_PAD - T_LOC, np.float32)
    vg[T_NO - 1 : T_NO - 1 + T] = V
    vg = vg.astype(BF16)

    halo = T_NO - 1
    ez = np.zeros((halo, S_e.shape[1]), np.float32)
    iz = np.zeros((halo, S_i.shape[1]), np.float32)
    in_maps = []
    for m in range(N_CORES):
        r0 = m * T_LOC
        if m == 0:
            se_m = np.concatenate([ez, S_e[:T_LOC]], 0)
            si_m = np.concatenate([iz, S_i[:T_LOC]], 0)
        else:
            se_m = S_e[r0 - halo : r0 + T_LOC]
            si_m = S_i[r0 - halo : r0 + T_LOC]
        in_maps.append(
            {"se": se_m, "si": si_m, "v": vg[r0 : r0 + V_LEN], **wd}
        )

    nc = _build(T_PAD, SE_ROWS, S_e.shape[1], S_i.shape[1], HID, SUB)
    trace = os.environ.get("CC_TRACE") == "1"
    res = run_bass_kernel_spmd(nc, in_maps, list(range(N_CORES)), trace=trace)
    LAST["exec_time_ns"] = res.exec_time_ns
    LAST["results"] = res
    out = np.concatenate(
        [res.results[m]["out"][:, :T_LOC].T for m in range(N_CORES)], 0
    )
    return np.ascontiguousarray(out.astype(np.float32))


# revision 8
# speedup vs baseline: 2.5513x; 1.2012x over previous
"""Trainium2 Bass kernel for nn_Encoder (sliding-window MLP + synaptic conv).

Computation (per timestep t of T_data):
  syn_e[t] = sum(S_e[t, :]);  syn_i[t] = sum(S_i[t, :])
  syn_out[t, s] = sum_k e_kern[s, k] * syn_e[t-k] + i_kern[s, k] * syn_i[t-k]
  Vw[t, :] = V[t-199 : t+200]   (zero padded)
  h = lrelu(Vw @ W1.T + b1); h = lrelu(h @ W2.T + b2); h = lrelu(h @ W3.T + b3)
  out[t, :] = tanh(h @ W4.T + b4 + syn_out[t, :])

Strategy: data-parallel over T across 8 NeuronCores, each core gets its
T/8 slice plus a 199-row halo of S_e/S_i and a 398-elem halo of V (host
zero-pads the edges).  On each core:
  - S_e/S_i row-sums via VectorE free-axis reduce (fp32), PE-transposed and
    cast to bf16, stored contiguously to a DRAM scratch vector.
  - All matmuls in bf16 (fp32 PSUM accumulate).  Sliding windows of V and
    syn_e/syn_i are materialized as matmul operands directly by overlapping
    ("Hankel") DMA access patterns from DRAM: partition step 1, free step 1.
    One wide Hankel tile per block serves all K-chunks as column slices.
  - Layers 1-3 keep hid on PSUM partitions; layer 4 + conv keep the
    [sub, t] orientation (weights stationary) so the output store is a
    contiguous [sub, nt] tile; the host transposes the gathered output.
    b4 is added via a K=1 matmul (b4 stationary, ones streaming).
  - lrelu/tanh run on ScalarE (Lrelu alpha=0.01, Tanh); both live in the
    same ACT LUT table set so only one table load is emitted.
"""

import os
from contextlib import ExitStack

import ml_dtypes
import numpy as np

import concourse.bass as bass
import concourse.mybir as mybir
import concourse.tile as tile
from concourse import bacc
from concourse.bass_utils import run_bass_kernel_spmd
from concourse.masks import make_identity
from concourse.tile_rust import add_dep_helper

BF16 = ml_dtypes.bfloat16
FP32 = mybir.dt.float32
BF = mybir.dt.bfloat16

T_NO = 200
WIN = 2 * T_NO - 1  # 399
N_CORES = 8
BLK = 512  # timesteps per block (one PSUM bank of fp32)

LAST = {}  # exec_time_ns / trace info from the most recent run (for test harness)


def _ceil_div(a, b):
    return -(-a // b)


def _chunks(total, step=128):
    out = []
    o = 0
    while o < total:
        out.append((o, min(step, total - o)))
        o += step
    return out


def _build(T_PAD, SE_ROWS, E_COLS, I_COLS, HID, SUB):
    """Build the per-core Bass program (identical on all 8 cores)."""
    R_TILES = _ceil_div(SE_ROWS, 128)
    # +128 margin: the wide hankel DMA loads a full [128, nt+W] rectangle
    # whose unused corner reads past the logical end
    SCR_LEN = R_TILES * 128 + 128
    V_LEN = T_PAD + WIN - 1 + 128
    NB = _ceil_div(T_PAD, BLK)

    m_hid = _chunks(HID)  # M chunks of hid (PSUM partitions L1-3)
    k_win = _chunks(WIN)  # K chunks of the V window
    k_hid = _chunks(HID)  # K chunks of hid (L2-4 contraction)
    k_syn = _chunks(T_NO)  # K chunks of the conv kernel length
    VH_W = 128 * (len(k_win) - 1)  # extra hankel cols so K-chunks are slices
    SY_W = 128 * (len(k_syn) - 1)

    nc = bacc.Bacc(
        "TRN2", target_bir_lowering=False, debug=False, num_devices=N_CORES
    )

    se_h = nc.dram_tensor("se", [SE_ROWS, E_COLS], FP32, kind="ExternalInput")
    si_h = nc.dram_tensor("si", [SE_ROWS, I_COLS], FP32, kind="ExternalInput")
    v_h = nc.dram_tensor("v", [V_LEN], BF, kind="ExternalInput")
    n_kw, n_kh, n_ks = len(k_win), len(k_hid), len(k_syn)
    w1t_h = nc.dram_tensor("w1t", [128, n_kw * HID], BF, kind="ExternalInput")
    w2t_h = nc.dram_tensor("w2t", [128, n_kh * HID], BF, kind="ExternalInput")
    w3t_h = nc.dram_tensor("w3t", [128, n_kh * HID], BF, kind="ExternalInput")
    b1_h = nc.dram_tensor("bpk", [128, 3 * n_kh], FP32, kind="ExternalInput")
    b4_h = nc.dram_tensor("b4", [SUB], BF, kind="ExternalInput")
    ekm_h = nc.dram_tensor("spk", [128, (n_kh + 2 * n_ks) * SUB], BF,
                           kind="ExternalInput")
    out_h = nc.dram_tensor("out", [SUB, T_PAD], FP32, kind="ExternalOutput")

    sse_h = nc.dram_tensor("sse_scratch", [SCR_LEN], BF)
    ssi_h = nc.dram_tensor("ssi_scratch", [SCR_LEN], BF)

    with tile.TileContext(nc) as tc, ExitStack() as ctx:
        cpool = ctx.enter_context(tc.tile_pool(name="consts", bufs=1))
        sepool = ctx.enter_context(tc.tile_pool(name="sein", bufs=8))
        accpool = ctx.enter_context(tc.tile_pool(name="acc", bufs=1))
        stpool = ctx.enter_context(tc.tile_pool(name="store", bufs=2))
        hkpool = ctx.enter_context(tc.tile_pool(name="hankel", bufs=2))
        hpool = ctx.enter_context(tc.tile_pool(name="acts", bufs=2))
        opool = ctx.enter_context(tc.tile_pool(name="outs", bufs=3))
        psmm = ctx.enter_context(tc.tile_pool(name="psmm", bufs=5, space="PSUM"))
        ps4p = ctx.enter_context(tc.tile_pool(name="ps4p", bufs=2, space="PSUM"))
        ptrp = ctx.enter_context(tc.tile_pool(name="ptrp", bufs=1, space="PSUM"))

        # ---- constants to SBUF (host-packed: one wide DMA per group) ----
        # w1t/w2t/w3t arrive packed as [128, nchunks*HID]: K-chunk kc of the
        # pre-transposed weight lives at columns [HID*kc, HID*(kc+1))
        def packed_w(dram, nm, ncols):
            t = cpool.tile([128, ncols], BF, name=nm, tag=nm)
            nc.sync.dma_start(out=t[:, :], in_=dram[:, :])
            return t

        w1t_pk = packed_w(w1t_h, "w1t", len(k_win) * HID)
        w2t_pk = packed_w(w2t_h, "w2t", len(k_hid) * HID)
        w3t_pk = packed_w(w3t_h, "w3t", len(k_hid) * HID)
        w1t_sb = [w1t_pk[:, HID * kc : HID * (kc + 1)] for kc in range(len(k_win))]
        w2t_sb = [w2t_pk[:, HID * kc : HID * (kc + 1)] for kc in range(len(k_hid))]
        w3t_sb = [w3t_pk[:, HID * kc : HID * (kc + 1)] for kc in range(len(k_hid))]
        # small pack: [w4t chunks | ek chunks | ik chunks] as [128, (4+2+2)*SUB]
        sp = cpool.tile([128, (len(k_hid) + 2 * len(k_syn)) * SUB], BF,
                        name="smallpk", tag="smallpk")
        nc.sync.dma_start(out=sp[:, :], in_=ekm_h[:, :])
        w4t_sb = [sp[:, SUB * kc : SUB * (kc + 1)] for kc in range(len(k_hid))]
        o1 = len(k_hid)
        ek_sb = [sp[:, SUB * (o1 + j) : SUB * (o1 + j + 1)] for j in range(len(k_syn))]
        o2 = o1 + len(k_syn)
        ik_sb = [sp[:, SUB * (o2 + j) : SUB * (o2 + j + 1)] for j in range(len(k_syn))]
        # biases packed [128, 3*nchunks] f32
        bp = cpool.tile([128, 3 * len(m_hid)], FP32, name="biaspk", tag="biaspk")
        nc.sync.dma_start(out=bp[:, :], in_=b1_h[:, :])
        bias_sb = {f"b{li + 1}": bp[:, li * len(m_hid) : (li + 1) * len(m_hid)]
                   for li in range(3)}

        b4_sb = cpool.tile([1, SUB], BF, name="b4sb", tag="b4sb")
        b4_dma = nc.sync.dma_start(
            out=b4_sb[0:1, :], in_=bass.AP(b4_h, 0, [[0, 1], [1, SUB]]))
        ones_sb = cpool.tile([1, BLK], BF, name="ones", tag="ones")
        nc.vector.memset(ones_sb[0:1, :], 1.0)
        ident = cpool.tile([128, 128], FP32, name="ident", tag="ident")
        make_identity(nc, ident[:, :])

        # ---- reduction accumulators ----
        se_acc = accpool.tile([128, R_TILES], FP32, name="se_acc", tag="se_acc")
        si_acc = accpool.tile([128, R_TILES], FP32, name="si_acc", tag="si_acc")
        if SE_ROWS % 128 != 0:
            # rows past SE_ROWS in the last reduce tile are never written by
            # the reduce; zero them so the scratch tail holds no garbage
            nc.vector.memset(se_acc[:, R_TILES - 1 : R_TILES], 0.0)
            nc.vector.memset(si_acc[:, R_TILES - 1 : R_TILES], 0.0)

        reduced = 0  # reduce tiles emitted so far
        stored = 0  # scratch columns stored so far

        def emit_reduce(i):
            r0 = 128 * i
            nr = min(128, SE_ROWS - r0)
            se_t = sepool.tile([128, E_COLS], FP32, name="se_t", tag="se_t")
            first = nc.gpsimd.dma_start(out=se_t[:nr, :], in_=se_h[r0 : r0 + nr, :])
            nc.vector.reduce_sum(se_acc[:nr, i : i + 1], se_t[:nr, :],
                                 axis=mybir.AxisListType.X)
            si_t = sepool.tile([128, I_COLS], FP32, name="si_t", tag="si_t")
            nc.gpsimd.dma_start(out=si_t[:nr, :], in_=si_h[r0 : r0 + nr, :])
            nc.vector.reduce_sum(si_acc[:nr, i : i + 1], si_t[:nr, :],
                                 axis=mybir.AxisListType.X)
            return first

        def emit_store(a, b):
            # PE-transpose fp32 accumulator cols [a,b) to [w,128], cast to
            # bf16 on ScalarE, store contiguously to the scratch vector
            w = b - a
            for nm, acc, scr in (("se", se_acc, sse_h), ("si", si_acc, ssi_h)):
                tr_t = ptrp.tile([16, 128], FP32, name=f"{nm}tr", tag="tr")
                nc.tensor.transpose(tr_t[:w, :], acc[:, a:b], ident[:, :])
                st_t = stpool.tile([16, 128], BF, name=f"{nm}st", tag=f"{nm}st")
                nc.scalar.activation(st_t[:w, :], tr_t[:w, :],
                                     mybir.ActivationFunctionType.Copy)
                nc.scalar.dma_start(
                    out=bass.AP(scr, 128 * a, [[128, w], [1, 128]]),
                    in_=st_t[:w, :],
                )

        # ---- main loop over timestep block pairs ----
        # Blocks are processed in pairs, interleaved at layer granularity:
        # L1(a) L1(b) L2(a) L2(b) ... so the PE never waits for the ScalarE
        # PSUM evacuation of the previous layer (it runs under the other
        # block's matmuls) and stays HAM-warm.  One wide Hankel tile per
        # pair serves both blocks and all K-chunks as column slices.
        pairs = [tuple(b for b in (i, i + 1) if b < NB) for i in range(0, NB, 2)]
        for pair in pairs:
            t0p = BLK * pair[0]
            blks = []
            off = 0
            for b in pair:
                nt = min(BLK, T_PAD - BLK * b)
                blks.append((BLK * b, nt, off))
                off += nt
            tot = off

            last_t0, last_nt, _ = blks[-1]
            need = min(R_TILES, _ceil_div(last_t0 + last_nt + T_NO - 1, 128))
            if pair is pairs[-1]:
                need = R_TILES

            vh = hkpool.tile([128, 2 * BLK + VH_W], BF, name="vh", tag="vh")
            vh_dma = nc.sync.dma_start(
                out=vh[:, : tot + VH_W],
                in_=bass.AP(v_h, t0p, [[1, 128], [1, tot + VH_W]]),
            )
            # emit the bulk reduce loads one pair ahead so the scratch-store
            # chain before the conv never waits on a reduce
            ahead = min(len(pairs) - 1, pairs.index(pair) + 1)
            la_t0, la_nt, _ = (
                blks[-1] if pair is pairs[-1] else None) or (0, 0, 0)
            tgt = R_TILES if ahead == len(pairs) - 1 or pair is pairs[-1] else \
                min(R_TILES, _ceil_div(BLK * (pairs[ahead][-1] + 1) + T_NO - 1, 128))
            tgt = max(tgt, need)
            while reduced < tgt:
                first = emit_reduce(reduced)
                if reduced == 0:
                    # let the startup-critical weight/hankel loads win the
                    # fabric before the bulk stream starts
                    add_dep_helper(first.ins, vh_dma.ins, sync=True,
                                   reason="gate bulk stream on startup loads")
                reduced += 1

            # layers 1..3 (hid on PSUM partitions), block-pair interleaved
            h_prev = {}  # (layer, block-slot) -> list of 4 sbuf tiles
            for lidx, (k_list, w_sb, bias_t) in enumerate((
                (k_win, w1t_sb, bias_sb["b1"]),
                (k_hid, w2t_sb, bias_sb["b2"]),
                (k_hid, w3t_sb, bias_sb["b3"]),
            )):
                for si_, (bt0, nt, coff) in enumerate(blks):
                    outs = []
                    for mc, (mo, nm_) in enumerate(m_hid):
                        ps = psmm.tile([128, BLK], FP32, name="ps", tag="ps")
                        for kc, (ko, pk) in enumerate(k_list):
                            if lidx == 0:
                                rhs = vh[:pk, coff + 128 * kc : coff + 128 * kc + nt]
                            else:
                                rhs = h_prev[si_][kc][:pk, :nt]
                            nc.tensor.matmul(
                                ps[:nm_, :nt],
                                w_sb[kc][:pk, mo : mo + nm_],
                                rhs,
                                start=(kc == 0),
                                stop=(kc == len(k_list) - 1),
                            )
                        h_t = hpool.tile([128, BLK], BF, name=f"h{lidx}_{mc}",
                                         tag=f"h{lidx}_{mc}")
                        nc.scalar.activation(
                            h_t[:nm_, :nt], ps[:nm_, :nt],
                            mybir.ActivationFunctionType.Lrelu,
                            bias=bias_t[:nm_, mc : mc + 1], alpha=0.01,
                        )
                        outs.append(h_t)
                    h_prev[si_] = outs

            # scratch stores (PE transpose) go here: their reduce inputs
            # completed during L1-3; the conv reads below depend on them
            while stored < need:
                emit_store(stored, min(need, stored + 16))
                stored = min(need, stored + 16)
            synh = {}
            for nm, scr in (("se", sse_h), ("si", ssi_h)):
                t = hkpool.tile([128, 2 * BLK + SY_W], BF, name=f"{nm}h",
                                tag=f"{nm}h")
                nc.sync.dma_start(
                    out=t[:, : tot + SY_W],
                    in_=bass.AP(scr, t0p, [[1, 128], [1, tot + SY_W]]),
                )
                synh[nm] = t

            # layer 4 + conv in [sub, t] orientation; b4 via K=1 matmul
            for si_, (bt0, nt, coff) in enumerate(blks):
                h3 = h_prev[si_]
                ps4 = ps4p.tile([SUB, BLK], FP32, name="ps4", tag="ps4")
                nc.tensor.matmul(ps4[:, :nt], b4_sb[0:1, :], ones_sb[0:1, :nt],
                                 start=True, stop=False)
                for kc, (ko, pk) in enumerate(k_hid):
                    nc.tensor.matmul(ps4[:, :nt], w4t_sb[kc][:pk, :],
                                     h3[kc][:pk, :nt], start=False, stop=False)
                for nm, k_sb in (("se", ek_sb), ("si", ik_sb)):
                    for j, (o, pk) in enumerate(k_syn):
                        last = nm == "si" and j == len(k_syn) - 1
                        nc.tensor.matmul(
                            ps4[:, :nt], k_sb[j][:pk, :],
                            synh[nm][:pk, coff + 128 * j : coff + 128 * j + nt],
                            start=False, stop=last,
                        )
                out_sb = opool.tile([SUB, BLK], FP32, name="out_sb", tag="out_sb")
                nc.scalar.activation(out_sb[:, :nt], ps4[:, :nt],
                                     mybir.ActivationFunctionType.Tanh)
                nc.sync.dma_start(out=out_h[:, bt0 : bt0 + nt], in_=out_sb[:, :nt])

    nc.compile()
    return nc


def kernel(V, S_e, S_i, W1, b1, W2, b2, W3, b3, W4, b4, W_syn, Tau_syn, Delta_syn):
    V = np.asarray(V, np.float32)
    S_e = np.ascontiguousarray(np.asarray(S_e, np.float32))
    S_i = np.ascontiguousarray(np.asarray(S_i, np.float32))
    T = V.shape[0]
    assert T % N_CORES == 0
    T_LOC = T // N_CORES
    T_PAD = _ceil_div(T_LOC, 128) * 128
    SE_ROWS = T_NO - 1 + T_LOC
    V_LEN = T_PAD + WIN - 1 + 128
    HID = W1.shape[0]
    SUB = W4.shape[0]

    # ---- tiny host-side prep (layout/dtype only + 20x200 conv kernels) ----
    W_syn = np.asarray(W_syn, np.float32)
    Tau_syn = np.asarray(Tau_syn, np.float32)
    Delta_syn = np.asarray(Delta_syn, np.float32)
    t_raw = np.arange(T_NO, dtype=np.float32)[None, :]
    t_e = np.maximum(t_raw - Delta_syn[:, 0:1], 0.0)
    t_i = np.maximum(t_raw - Delta_syn[:, 1:2], 0.0)
    tt_e = t_e / Tau_syn[:, 0:1] ** 2
    tt_i = t_i / Tau_syn[:, 1:2] ** 2
    e_kern = tt_e * np.exp(-tt_e) * W_syn[:, 0:1] ** 2
    i_kern = -(tt_i * np.exp(-tt_i)) * W_syn[:, 1:2] ** 2
    ekm = np.ascontiguousarray(e_kern[:, ::-1].T).astype(BF16)  # [T_NO, SUB]
    ikm = np.ascontiguousarray(i_kern[:, ::-1].T).astype(BF16)

    def pack_rows(mat, nch):
        # [R, C] -> [128, nch*C]: chunk kc rows at columns [C*kc, C*(kc+1))
        r, c = mat.shape
        out = np.zeros((128, nch * c), np.float32)
        for kc in range(nch):
            rows = mat[128 * kc : min(r, 128 * (kc + 1))]
            out[: rows.shape[0], c * kc : c * kc + c] = rows
        return out

    w1t = np.asarray(W1, np.float32).T
    w2t = np.asarray(W2, np.float32).T
    w3t = np.asarray(W3, np.float32).T
    w4t = np.asarray(W4, np.float32).T
    n_kw, n_kh, n_ks = _ceil_div(WIN, 128), _ceil_div(HID, 128), _ceil_div(T_NO, 128)
    spk = np.concatenate(
        [pack_rows(w4t, n_kh), pack_rows(ekm.astype(np.float32), n_ks),
         pack_rows(ikm.astype(np.float32), n_ks)], 1)
    bpk = np.concatenate(
        [pack_rows(np.asarray(b, np.float32)[:, None], n_kh).reshape(128, n_kh)
         for b in (b1, b2, b3)], 1)
    wd = {
        "w1t": pack_rows(w1t, n_kw).astype(BF16),
        "w2t": pack_rows(w2t, n_kh).astype(BF16),
        "w3t": pack_rows(w3t, n_kh).astype(BF16),
        "bpk": np.ascontiguousarray(bpk, np.float32),
        "b4": np.asarray(b4, np.float32).astype(BF16),
        "spk": spk.astype(BF16),
    }

    vg = np.zeros(T_NO - 1 + T + WIN + 128 + T_PAD - T_LOC, np.float32)
    vg[T_NO - 1 : T_NO - 1 + T] = V
    vg = vg.astype(BF16)

    halo = T_NO - 1
    ez = np.zeros((halo, S_e.shape[1]), np.float32)
    iz = np.zeros((halo, S_i.shape[1]), np.float32)
    in_maps = []
    for m in range(N_CORES):
        r0 = m * T_LOC
        if m == 0:
            se_m = np.concatenate([ez, S_e[:T_LOC]], 0)
            si_m = np.concatenate([iz, S_i[:T_LOC]], 0)
        else:
            se_m = S_e[r0 - halo : r0 + T_LOC]
            si_m = S_i[r0 - halo : r0 + T_LOC]
        in_maps.append(
            {"se": se_m, "si": si_m, "v": vg[r0 : r0 + V_LEN], **wd}
        )

    nc = _build(T_PAD, SE_ROWS, S_e.shape[1], S_i.shape[1], HID, SUB)
    trace = os.environ.get("CC_TRACE") == "1"
    res = run_bass_kernel_spmd(nc, in_maps, list(range(N_CORES)), trace=trace)
    LAST["exec_time_ns"] = res.exec_time_ns
    LAST["results"] = res
    out = np.concatenate(
        [res.results[m]["out"][:, :T_LOC].T for m in range(N_CORES)], 0
    )
    return np.ascontiguousarray(out.astype(np.float32))


# revision 9
# speedup vs baseline: 2.5941x; 1.0168x over previous
"""Trainium2 Bass kernel for nn_Encoder (sliding-window MLP + synaptic conv).

Computation (per timestep t of T_data):
  syn_e[t] = sum(S_e[t, :]);  syn_i[t] = sum(S_i[t, :])
  syn_out[t, s] = sum_k e_kern[s, k] * syn_e[t-k] + i_kern[s, k] * syn_i[t-k]
  Vw[t, :] = V[t-199 : t+200]   (zero padded)
  h = lrelu(Vw @ W1.T + b1); h = lrelu(h @ W2.T + b2); h = lrelu(h @ W3.T + b3)
  out[t, :] = tanh(h @ W4.T + b4 + syn_out[t, :])

Strategy: data-parallel over T across 8 NeuronCores, each core gets its
T/8 slice plus a 199-row halo of S_e/S_i and a 398-elem halo of V (host
zero-pads the edges).  On each core:
  - S_e/S_i row-sums via VectorE free-axis reduce (fp32), PE-transposed and
    cast to bf16, stored contiguously to a DRAM scratch vector.
  - All matmuls in bf16 (fp32 PSUM accumulate).  Sliding windows of V and
    syn_e/syn_i are materialized as matmul operands directly by overlapping
    ("Hankel") DMA access patterns from DRAM: partition step 1, free step 1.
    One wide Hankel tile per block serves all K-chunks as column slices.
  - Layers 1-3 keep hid on PSUM partitions; layer 4 + conv keep the
    [sub, t] orientation (weights stationary) so the output store is a
    contiguous [sub, nt] tile; the host transposes the gathered output.
    b4 is added via a K=1 matmul (b4 stationary, ones streaming).
  - lrelu/tanh run on ScalarE (Lrelu alpha=0.01, Tanh); both live in the
    same ACT LUT table set so only one table load is emitted.
"""

import os
from contextlib import ExitStack

import ml_dtypes
import numpy as np

import concourse.bass as bass
import concourse.mybir as mybir
import concourse.tile as tile
from concourse import bacc
from concourse.bass_utils import run_bass_kernel_spmd
from concourse.masks import make_identity
from concourse.tile_rust import add_dep_helper

BF16 = ml_dtypes.bfloat16
FP32 = mybir.dt.float32
BF = mybir.dt.bfloat16

T_NO = 200
WIN = 2 * T_NO - 1  # 399
N_CORES = 8
BLK = 512  # timesteps per block (one PSUM bank of fp32)

LAST = {}  # exec_time_ns / trace info from the most recent run (for test harness)


def _ceil_div(a, b):
    return -(-a // b)


def _chunks(total, step=128):
    out = []
    o = 0
    while o < total:
        out.append((o, min(step, total - o)))
        o += step
    return out


def _build(T_PAD, SE_ROWS, E_COLS, I_COLS, HID, SUB):
    """Build the per-core Bass program (identical on all 8 cores)."""
    R_TILES = _ceil_div(SE_ROWS, 128)
    # +128 margin: the wide hankel DMA loads a full [128, nt+W] rectangle
    # whose unused corner reads past the logical end
    SCR_LEN = R_TILES * 128 + 128
    V_LEN = T_PAD + WIN - 1 + 128
    NB = _ceil_div(T_PAD, BLK)

    m_hid = _chunks(HID)  # M chunks of hid (PSUM partitions L1-3)
    k_win = _chunks(WIN)  # K chunks of the V window
    k_hid = _chunks(HID)  # K chunks of hid (L2-4 contraction)
    k_syn = _chunks(T_NO)  # K chunks of the conv kernel length
    VH_W = 128 * (len(k_win) - 1)  # extra hankel cols so K-chunks are slices
    SY_W = 128 * (len(k_syn) - 1)

    nc = bacc.Bacc(
        "TRN2", target_bir_lowering=False, debug=False, num_devices=N_CORES
    )

    se_h = nc.dram_tensor("se", [SE_ROWS, E_COLS], FP32, kind="ExternalInput")
    si_h = nc.dram_tensor("si", [SE_ROWS, I_COLS], FP32, kind="ExternalInput")
    v_h = nc.dram_tensor("v", [V_LEN], BF, kind="ExternalInput")
    n_kw, n_kh, n_ks = len(k_win), len(k_hid), len(k_syn)
    w1t_h = nc.dram_tensor("w1t", [128, n_kw * HID], BF, kind="ExternalInput")
    w2t_h = nc.dram_tensor("w2t", [128, n_kh * HID], BF, kind="ExternalInput")
    w3t_h = nc.dram_tensor("w3t", [128, n_kh * HID], BF, kind="ExternalInput")
    b1_h = nc.dram_tensor("bpk", [128, 3 * n_kh], FP32, kind="ExternalInput")
    b4_h = nc.dram_tensor("b4", [SUB], BF, kind="ExternalInput")
    ekm_h = nc.dram_tensor("spk", [128, (n_kh + 2 * n_ks) * SUB], BF,
                           kind="ExternalInput")
    out_h = nc.dram_tensor("out", [SUB, T_PAD], FP32, kind="ExternalOutput")

    sse_h = nc.dram_tensor("sse_scratch", [SCR_LEN], BF)
    ssi_h = nc.dram_tensor("ssi_scratch", [SCR_LEN], BF)

    with tile.TileContext(nc) as tc, ExitStack() as ctx:
        cpool = ctx.enter_context(tc.tile_pool(name="consts", bufs=1))
        sepool = ctx.enter_context(tc.tile_pool(name="sein", bufs=8))
        accpool = ctx.enter_context(tc.tile_pool(name="acc", bufs=1))
        stpool = ctx.enter_context(tc.tile_pool(name="store", bufs=2))
        hkpool = ctx.enter_context(tc.tile_pool(name="hankel", bufs=3))
        hpool = ctx.enter_context(tc.tile_pool(name="acts", bufs=2))
        opool = ctx.enter_context(tc.tile_pool(name="outs", bufs=3))
        psmm = ctx.enter_context(tc.tile_pool(name="psmm", bufs=5, space="PSUM"))
        ps4p = ctx.enter_context(tc.tile_pool(name="ps4p", bufs=2, space="PSUM"))
        ptrp = ctx.enter_context(tc.tile_pool(name="ptrp", bufs=1, space="PSUM"))

        # ---- constants to SBUF (host-packed: one wide DMA per group) ----
        # w1t/w2t/w3t arrive packed as [128, nchunks*HID]: K-chunk kc of the
        # pre-transposed weight lives at columns [HID*kc, HID*(kc+1))
        def packed_w(dram, nm, ncols):
            t = cpool.tile([128, ncols], BF, name=nm, tag=nm)
            nc.sync.dma_start(out=t[:, :], in_=dram[:, :])
            return t

        w1t_pk = packed_w(w1t_h, "w1t", len(k_win) * HID)
        w2t_pk = packed_w(w2t_h, "w2t", len(k_hid) * HID)
        w3t_pk = packed_w(w3t_h, "w3t", len(k_hid) * HID)
        w1t_sb = [w1t_pk[:, HID * kc : HID * (kc + 1)] for kc in range(len(k_win))]
        w2t_sb = [w2t_pk[:, HID * kc : HID * (kc + 1)] for kc in range(len(k_hid))]
        w3t_sb = [w3t_pk[:, HID * kc : HID * (kc + 1)] for kc in range(len(k_hid))]
        # small pack: [w4t chunks | ek chunks | ik chunks] as [128, (4+2+2)*SUB]
        sp = cpool.tile([128, (len(k_hid) + 2 * len(k_syn)) * SUB], BF,
                        name="smallpk", tag="smallpk")
        nc.sync.dma_start(out=sp[:, :], in_=ekm_h[:, :])
        w4t_sb = [sp[:, SUB * kc : SUB * (kc + 1)] for kc in range(len(k_hid))]
        o1 = len(k_hid)
        ek_sb = [sp[:, SUB * (o1 + j) : SUB * (o1 + j + 1)] for j in range(len(k_syn))]
        o2 = o1 + len(k_syn)
        ik_sb = [sp[:, SUB * (o2 + j) : SUB * (o2 + j + 1)] for j in range(len(k_syn))]
        # biases packed [128, 3*nchunks] f32
        bp = cpool.tile([128, 3 * len(m_hid)], FP32, name="biaspk", tag="biaspk")
        nc.sync.dma_start(out=bp[:, :], in_=b1_h[:, :])
        bias_sb = {f"b{li + 1}": bp[:, li * len(m_hid) : (li + 1) * len(m_hid)]
                   for li in range(3)}

        b4_sb = cpool.tile([1, SUB], BF, name="b4sb", tag="b4sb")
        b4_dma = nc.sync.dma_start(
            out=b4_sb[0:1, :], in_=bass.AP(b4_h, 0, [[0, 1], [1, SUB]]))
        ones_sb = cpool.tile([1, BLK], BF, name="ones", tag="ones")
        nc.vector.memset(ones_sb[0:1, :], 1.0)
        ident = cpool.tile([128, 128], FP32, name="ident", tag="ident")
        make_identity(nc, ident[:, :])

        # ---- reduction accumulators ----
        se_acc = accpool.tile([128, R_TILES], FP32, name="se_acc", tag="se_acc")
        si_acc = accpool.tile([128, R_TILES], FP32, name="si_acc", tag="si_acc")
        if SE_ROWS % 128 != 0:
            # rows past SE_ROWS in the last reduce tile are never written by
            # the reduce; zero them so the scratch tail holds no garbage
            nc.vector.memset(se_acc[:, R_TILES - 1 : R_TILES], 0.0)
            nc.vector.memset(si_acc[:, R_TILES - 1 : R_TILES], 0.0)

        reduced = 0  # reduce tiles emitted so far
        stored = 0  # scratch columns stored so far

        def emit_reduce(i):
            r0 = 128 * i
            nr = min(128, SE_ROWS - r0)
            se_t = sepool.tile([128, E_COLS], FP32, name="se_t", tag="se_t")
            first = nc.gpsimd.dma_start(out=se_t[:nr, :], in_=se_h[r0 : r0 + nr, :])
            nc.vector.reduce_sum(se_acc[:nr, i : i + 1], se_t[:nr, :],
                                 axis=mybir.AxisListType.X)
            si_t = sepool.tile([128, I_COLS], FP32, name="si_t", tag="si_t")
            nc.gpsimd.dma_start(out=si_t[:nr, :], in_=si_h[r0 : r0 + nr, :])
            nc.vector.reduce_sum(si_acc[:nr, i : i + 1], si_t[:nr, :],
                                 axis=mybir.AxisListType.X)
            return first

        def emit_store(a, b):
            # PE-transpose fp32 accumulator cols [a,b) to [w,128], cast to
            # bf16 on ScalarE, store contiguously to the scratch vector
            w = b - a
            for nm, acc, scr in (("se", se_acc, sse_h), ("si", si_acc, ssi_h)):
                tr_t = ptrp.tile([16, 128], FP32, name=f"{nm}tr", tag="tr")
                nc.tensor.transpose(tr_t[:w, :], acc[:, a:b], ident[:, :])
                st_t = stpool.tile([16, 128], BF, name=f"{nm}st", tag=f"{nm}st")
                nc.scalar.activation(st_t[:w, :], tr_t[:w, :],
                                     mybir.ActivationFunctionType.Copy)
                nc.scalar.dma_start(
                    out=bass.AP(scr, 128 * a, [[128, w], [1, 128]]),
                    in_=st_t[:w, :],
                )

        # ---- main loop over timestep block pairs ----
        # Blocks are processed in pairs, interleaved at layer granularity:
        # L1(a) L1(b) L2(a) L2(b) ... so the PE never waits for the ScalarE
        # PSUM evacuation of the previous layer (it runs under the other
        # block's matmuls) and stays HAM-warm.  One wide Hankel tile per
        # pair serves both blocks and all K-chunks as column slices.  The
        # conv operands are produced a full pair ahead (reduce loads two
        # pairs ahead) so the layer-4 conv never waits on the
        # reduce -> transpose -> store -> hankel-reload chain.
        pairs = [tuple(b for b in (i, i + 1) if b < NB) for i in range(0, NB, 2)]
        NP = len(pairs)
        pair_blks = []
        needs = []
        for pi, pair in enumerate(pairs):
            blks = []
            off = 0
            for b in pair:
                nt = min(BLK, T_PAD - BLK * b)
                blks.append((BLK * b, nt, off))
                off += nt
            pair_blks.append((blks, off))
            lt0, lnt, _ = blks[-1]
            needs.append(R_TILES if pi == NP - 1 else
                         min(R_TILES, _ceil_div(lt0 + lnt + T_NO - 1, 128)))

        synh_tiles = {}

        def emit_synh(pi):
            t0p = BLK * pairs[pi][0]
            tot = pair_blks[pi][1]
            synh = {}
            for nm, scr in (("se", sse_h), ("si", ssi_h)):
                t = hkpool.tile([128, 2 * BLK + SY_W], BF, name=f"{nm}h",
                                tag=f"{nm}h")
                nc.sync.dma_start(
                    out=t[:, : tot + SY_W],
                    in_=bass.AP(scr, t0p, [[1, 128], [1, tot + SY_W]]),
                )
                synh[nm] = t
            synh_tiles[pi] = synh

        def emit_stores_until(tgt):
            nonlocal stored
            while stored < tgt:
                emit_store(stored, min(tgt, stored + 16))
                stored = min(tgt, stored + 16)

        def emit_reduces_until(tgt, gate=None):
            nonlocal reduced
            while reduced < tgt:
                first = emit_reduce(reduced)
                if reduced == 0 and gate is not None:
                    # let the startup-critical weight/hankel loads win the
                    # fabric before the bulk stream starts
                    add_dep_helper(first.ins, gate.ins, sync=True,
                                   reason="gate bulk stream on startup loads")
                reduced += 1

        for pi, pair in enumerate(pairs):
            blks, tot = pair_blks[pi]
            t0p = BLK * pair[0]

            vh = hkpool.tile([128, 2 * BLK + VH_W], BF, name="vh", tag="vh")
            vh_dma = nc.sync.dma_start(
                out=vh[:, : tot + VH_W],
                in_=bass.AP(v_h, t0p, [[1, 128], [1, tot + VH_W]]),
            )
            if pi == 0:
                emit_reduces_until(needs[0], gate=vh_dma)
                emit_reduces_until(needs[min(1, NP - 1)])

            # layers 1..3 (hid on PSUM partitions), block-pair interleaved
            h_prev = {}
            for lidx, (k_list, w_sb, bias_t) in enumerate((
                (k_win, w1t_sb, bias_sb["b1"]),
                (k_hid, w2t_sb, bias_sb["b2"]),
                (k_hid, w3t_sb, bias_sb["b3"]),
            )):
                for si_, (bt0, nt, coff) in enumerate(blks):
                    outs = []
                    for mc, (mo, nm_) in enumerate(m_hid):
                        ps = psmm.tile([128, BLK], FP32, name="ps", tag="ps")
                        for kc, (ko, pk) in enumerate(k_list):
                            if lidx == 0:
                                rhs = vh[:pk, coff + 128 * kc : coff + 128 * kc + nt]
                            else:
                                rhs = h_prev[si_][kc][:pk, :nt]
                            nc.tensor.matmul(
                                ps[:nm_, :nt],
                                w_sb[kc][:pk, mo : mo + nm_],
                                rhs,
                                start=(kc == 0),
                                stop=(kc == len(k_list) - 1),
                            )
                        h_t = hpool.tile([128, BLK], BF, name=f"h{lidx}_{mc}",
                                         tag=f"h{lidx}_{mc}")
                        nc.scalar.activation(
                            h_t[:nm_, :nt], ps[:nm_, :nt],
                            mybir.ActivationFunctionType.Lrelu,
                            bias=bias_t[:nm_, mc : mc + 1], alpha=0.01,
                        )
                        outs.append(h_t)
                    h_prev[si_] = outs

            # conv operand prefetch: this pair's (pair 0) or the next pair's
            # scratch stores + hankel reloads; transposes land in the PE
            # queue here, between L3 and the conv matmuls
            if pi == 0:
                emit_stores_until(needs[0])
                emit_synh(0)
            if pi + 1 < NP:
                emit_stores_until(needs[pi + 1])
                emit_synh(pi + 1)

            # layer 4 + conv in [sub, t] orientation; b4 via K=1 matmul
            synh = synh_tiles.pop(pi)
            for si_, (bt0, nt, coff) in enumerate(blks):
                h3 = h_prev[si_]
                ps4 = ps4p.tile([SUB, BLK], FP32, name="ps4", tag="ps4")
                nc.tensor.matmul(ps4[:, :nt], b4_sb[0:1, :], ones_sb[0:1, :nt],
                                 start=True, stop=False)
                for kc, (ko, pk) in enumerate(k_hid):
                    nc.tensor.matmul(ps4[:, :nt], w4t_sb[kc][:pk, :],
                                     h3[kc][:pk, :nt], start=False, stop=False)
                for nm, k_sb in (("se", ek_sb), ("si", ik_sb)):
                    for j, (o, pk) in enumerate(k_syn):
                        last = nm == "si" and j == len(k_syn) - 1
                        nc.tensor.matmul(
                            ps4[:, :nt], k_sb[j][:pk, :],
                            synh[nm][:pk, coff + 128 * j : coff + 128 * j + nt],
                            start=False, stop=last,
                        )
                out_sb = opool.tile([SUB, BLK], FP32, name="out_sb", tag="out_sb")
                nc.scalar.activation(out_sb[:, :nt], ps4[:, :nt],
                                     mybir.ActivationFunctionType.Tanh)
                nc.sync.dma_start(out=out_h[:, bt0 : bt0 + nt], in_=out_sb[:, :nt])

            # bulk reduce loads two pairs ahead
            if pi + 2 < NP:
                emit_reduces_until(needs[pi + 2])

    nc.compile()
    return nc


def kernel(V, S_e, S_i, W1, b1, W2, b2, W3, b3, W4, b4, W_syn, Tau_syn, Delta_syn):
    V = np.asarray(V, np.float32)
    S_e = np.ascontiguousarray(np.asarray(S_e, np.float32))
    S_i = np.ascontiguousarray(np.asarray(S_i, np.float32))
    T = V.shape[0]
    assert T % N_CORES == 0
    T_LOC = T // N_CORES
    T_PAD = _ceil_div(T_LOC, 128) * 128
    SE_ROWS = T_NO - 1 + T_LOC
    V_LEN = T_PAD + WIN - 1 + 128
    HID = W1.shape[0]
    SUB = W4.shape[0]

    # ---- tiny host-side prep (layout/dtype only + 20x200 conv kernels) ----
    W_syn = np.asarray(W_syn, np.float32)
    Tau_syn = np.asarray(Tau_syn, np.float32)
    Delta_syn = np.asarray(Delta_syn, np.float32)
    t_raw = np.arange(T_NO, dtype=np.float32)[None, :]
    t_e = np.maximum(t_raw - Delta_syn[:, 0:1], 0.0)
    t_i = np.maximum(t_raw - Delta_syn[:, 1:2], 0.0)
    tt_e = t_e / Tau_syn[:, 0:1] ** 2
    tt_i = t_i / Tau_syn[:, 1:2] ** 2
    e_kern = tt_e * np.exp(-tt_e) * W_syn[:, 0:1] ** 2
    i_kern = -(tt_i * np.exp(-tt_i)) * W_syn[:, 1:2] ** 2
    ekm = np.ascontiguousarray(e_kern[:, ::-1].T).astype(BF16)  # [T_NO, SUB]
    ikm = np.ascontiguousarray(i_kern[:, ::-1].T).astype(BF16)

    def pack_rows(mat, nch):
        # [R, C] -> [128, nch*C]: chunk kc rows at columns [C*kc, C*(kc+1))
        r, c = mat.shape
        out = np.zeros((128, nch * c), np.float32)
        for kc in range(nch):
            rows = mat[128 * kc : min(r, 128 * (kc + 1))]
            out[: rows.shape[0], c * kc : c * kc + c] = rows
        return out

    w1t = np.asarray(W1, np.float32).T
    w2t = np.asarray(W2, np.float32).T
    w3t = np.asarray(W3, np.float32).T
    w4t = np.asarray(W4, np.float32).T
    n_kw, n_kh, n_ks = _ceil_div(WIN, 128), _ceil_div(HID, 128), _ceil_div(T_NO, 128)
    spk = np.concatenate(
        [pack_rows(w4t, n_kh), pack_rows(ekm.astype(np.float32), n_ks),
         pack_rows(ikm.astype(np.float32), n_ks)], 1)
    bpk = np.concatenate(
        [pack_rows(np.asarray(b, np.float32)[:, None], n_kh).reshape(128, n_kh)
         for b in (b1, b2, b3)], 1)
    wd = {
        "w1t": pack_rows(w1t, n_kw).astype(BF16),
        "w2t": pack_rows(w2t, n_kh).astype(BF16),
        "w3t": pack_rows(w3t, n_kh).astype(BF16),
        "bpk": np.ascontiguousarray(bpk, np.float32),
        "b4": np.asarray(b4, np.float32).astype(BF16),
        "spk": spk.astype(BF16),
    }

    vg = np.zeros(T_NO - 1 + T + WIN + 128 + T_PAD - T_LOC, np.float32)
    vg[T_NO - 1 : T_NO - 1 + T] = V
    vg = vg.astype(BF16)

    halo = T_NO - 1
    ez = np.zeros((halo, S_e.shape[1]), np.float32)
    iz = np.zeros((halo, S_i.shape[1]), np.float32)
    in_maps = []
    for m in range(N_CORES):
        r0 = m * T_LOC
        if m == 0:
            se_m = np.concatenate([ez, S_e[:T_LOC]], 0)
            si_m = np.concatenate([iz, S_i[:T_LOC]], 0)
        else:
            se_m = S_e[r0 - halo : r0 + T_LOC]
            si_m = S_i[r0 - halo : r0 + T_LOC]
        in_maps.append(
            {"se": se_m, "si": si_m, "v": vg[r0 : r0 + V_LEN], **wd}
        )

    nc = _build(T_PAD, SE_ROWS, S_e.shape[1], S_i.shape[1], HID, SUB)
    trace = os.environ.get("CC_TRACE") == "1"
    res = run_bass_kernel_spmd(nc, in_maps, list(range(N_CORES)), trace=trace)
    LAST["exec_time_ns"] = res.exec_time_ns
    LAST["results"] = res
    out = np.concatenate(
        [res.results[m]["out"][:, :T_LOC].T for m in range(N_CORES)], 0
    )
    return np.ascontiguousarray(out.astype(np.float32))


# revision 10
# speedup vs baseline: 2.6923x; 1.0379x over previous
"""Trainium2 Bass kernel for nn_Encoder (sliding-window MLP + synaptic conv).

Computation (per timestep t of T_data):
  syn_e[t] = sum(S_e[t, :]);  syn_i[t] = sum(S_i[t, :])
  syn_out[t, s] = sum_k e_kern[s, k] * syn_e[t-k] + i_kern[s, k] * syn_i[t-k]
  Vw[t, :] = V[t-199 : t+200]   (zero padded)
  h = lrelu(Vw @ W1.T + b1); h = lrelu(h @ W2.T + b2); h = lrelu(h @ W3.T + b3)
  out[t, :] = tanh(h @ W4.T + b4 + syn_out[t, :])

Strategy: data-parallel over T across 8 NeuronCores, each core gets its
T/8 slice plus a 199-row halo of S_e/S_i and a 398-elem halo of V (host
zero-pads the edges).  On each core:
  - S_e/S_i row-sums via VectorE free-axis reduce (fp32), PE-transposed and
    cast to bf16, stored contiguously to a DRAM scratch vector.
  - All matmuls in bf16 (fp32 PSUM accumulate).  Sliding windows of V and
    syn_e/syn_i are materialized as matmul operands directly by overlapping
    ("Hankel") DMA access patterns from DRAM: partition step 1, free step 1.
    One wide Hankel tile per block serves all K-chunks as column slices.
  - Layers 1-3 keep hid on PSUM partitions; layer 4 + conv keep the
    [sub, t] orientation (weights stationary) so the output store is a
    contiguous [sub, nt] tile; the host transposes the gathered output.
    b4 is added via a K=1 matmul (b4 stationary, ones streaming).
  - lrelu/tanh run on ScalarE (Lrelu alpha=0.01, Tanh); both live in the
    same ACT LUT table set so only one table load is emitted.
"""

import os
from contextlib import ExitStack

import ml_dtypes
import numpy as np

import concourse.bass as bass
import concourse.mybir as mybir
import concourse.tile as tile
from concourse import bacc
from concourse.bass_utils import run_bass_kernel_spmd
from concourse.masks import make_identity
from concourse.tile_rust import add_dep_helper

BF16 = ml_dtypes.bfloat16
FP32 = mybir.dt.float32
BF = mybir.dt.bfloat16

T_NO = 200
WIN = 2 * T_NO - 1  # 399
N_CORES = 8
BLK = 512  # timesteps per block (one PSUM bank of fp32)

LAST = {}  # exec_time_ns / trace info from the most recent run (for test harness)


def _ceil_div(a, b):
    return -(-a // b)


def _chunks(total, step=128):
    out = []
    o = 0
    while o < total:
        out.append((o, min(step, total - o)))
        o += step
    return out


def _build(T_PAD, SE_ROWS, E_COLS, I_COLS, HID, SUB):
    """Build the per-core Bass program (identical on all 8 cores)."""
    R_TILES = _ceil_div(SE_ROWS, 128)
    # +128 margin: the wide hankel DMA loads a full [128, nt+W] rectangle
    # whose unused corner reads past the logical end
    SCR_LEN = R_TILES * 128 + 128
    V_LEN = T_PAD + WIN - 1 + 128
    NB = _ceil_div(T_PAD, BLK)

    m_hid = _chunks(HID)  # M chunks of hid (PSUM partitions L1-3)
    k_win = _chunks(WIN)  # K chunks of the V window
    k_hid = _chunks(HID)  # K chunks of hid (L2-4 contraction)
    k_syn = _chunks(T_NO)  # K chunks of the conv kernel length
    VH_W = 128 * (len(k_win) - 1)  # extra hankel cols so K-chunks are slices
    SY_W = 128 * (len(k_syn) - 1)

    nc = bacc.Bacc(
        "TRN2", target_bir_lowering=False, debug=False, num_devices=N_CORES
    )

    se_h = nc.dram_tensor("se", [SE_ROWS, E_COLS], FP32, kind="ExternalInput")
    si_h = nc.dram_tensor("si", [SE_ROWS, I_COLS], FP32, kind="ExternalInput")
    v_h = nc.dram_tensor("v", [V_LEN], BF, kind="ExternalInput")
    n_kw, n_kh, n_ks = len(k_win), len(k_hid), len(k_syn)
    w1t_h = nc.dram_tensor("w1t", [128, n_kw * HID], BF, kind="ExternalInput")
    w2t_h = nc.dram_tensor("w2t", [128, n_kh * HID], BF, kind="ExternalInput")
    w3t_h = nc.dram_tensor("w3t", [128, n_kh * HID], BF, kind="ExternalInput")
    b1_h = nc.dram_tensor("bpk", [128, 3 * n_kh], FP32, kind="ExternalInput")
    b4_h = nc.dram_tensor("b4", [SUB], BF, kind="ExternalInput")
    ekm_h = nc.dram_tensor("spk", [128, (n_kh + 2 * n_ks) * SUB], BF,
                           kind="ExternalInput")
    out_h = nc.dram_tensor("out", [SUB, T_PAD], FP32, kind="ExternalOutput")

    sse_h = nc.dram_tensor("sse_scratch", [SCR_LEN], BF)
    ssi_h = nc.dram_tensor("ssi_scratch", [SCR_LEN], BF)

    with tile.TileContext(nc) as tc, ExitStack() as ctx:
        cpool = ctx.enter_context(tc.tile_pool(name="consts", bufs=1))
        sepool = ctx.enter_context(tc.tile_pool(name="sein", bufs=8))
        accpool = ctx.enter_context(tc.tile_pool(name="acc", bufs=1))
        stpool = ctx.enter_context(tc.tile_pool(name="store", bufs=2))
        hkpool = ctx.enter_context(tc.tile_pool(name="hankel", bufs=3))
        hpool = ctx.enter_context(tc.tile_pool(name="acts", bufs=2))
        opool = ctx.enter_context(tc.tile_pool(name="outs", bufs=3))
        psmm = ctx.enter_context(tc.tile_pool(name="psmm", bufs=5, space="PSUM"))
        ps4p = ctx.enter_context(tc.tile_pool(name="ps4p", bufs=2, space="PSUM"))
        ptrp = ctx.enter_context(tc.tile_pool(name="ptrp", bufs=1, space="PSUM"))

        # ---- constants to SBUF (host-packed: one wide DMA per group) ----
        # w1t/w2t/w3t arrive packed as [128, nchunks*HID]: K-chunk kc of the
        # pre-transposed weight lives at columns [HID*kc, HID*(kc+1))
        def packed_w(dram, nm, ncols):
            t = cpool.tile([128, ncols], BF, name=nm, tag=nm)
            nc.sync.dma_start(out=t[:, :], in_=dram[:, :])
            return t

        w1t_pk = packed_w(w1t_h, "w1t", len(k_win) * HID)
        w2t_pk = packed_w(w2t_h, "w2t", len(k_hid) * HID)
        w3t_pk = packed_w(w3t_h, "w3t", len(k_hid) * HID)
        w1t_sb = [w1t_pk[:, HID * kc : HID * (kc + 1)] for kc in range(len(k_win))]
        w2t_sb = [w2t_pk[:, HID * kc : HID * (kc + 1)] for kc in range(len(k_hid))]
        w3t_sb = [w3t_pk[:, HID * kc : HID * (kc + 1)] for kc in range(len(k_hid))]
        # small pack: [w4t chunks | ek chunks | ik chunks] as [128, (4+2+2)*SUB]
        sp = cpool.tile([128, (len(k_hid) + 2 * len(k_syn)) * SUB], BF,
                        name="smallpk", tag="smallpk")
        nc.sync.dma_start(out=sp[:, :], in_=ekm_h[:, :])
        w4t_sb = [sp[:, SUB * kc : SUB * (kc + 1)] for kc in range(len(k_hid))]
        o1 = len(k_hid)
        ek_sb = [sp[:, SUB * (o1 + j) : SUB * (o1 + j + 1)] for j in range(len(k_syn))]
        o2 = o1 + len(k_syn)
        ik_sb = [sp[:, SUB * (o2 + j) : SUB * (o2 + j + 1)] for j in range(len(k_syn))]
        # biases packed [128, 3*nchunks] f32
        bp = cpool.tile([128, 3 * len(m_hid)], FP32, name="biaspk", tag="biaspk")
        nc.sync.dma_start(out=bp[:, :], in_=b1_h[:, :])
        bias_sb = {f"b{li + 1}": bp[:, li * len(m_hid) : (li + 1) * len(m_hid)]
                   for li in range(3)}

        b4_sb = cpool.tile([1, SUB], BF, name="b4sb", tag="b4sb")
        b4_dma = nc.sync.dma_start(
            out=b4_sb[0:1, :], in_=bass.AP(b4_h, 0, [[0, 1], [1, SUB]]))
        ones_sb = cpool.tile([1, BLK], BF, name="ones", tag="ones")
        nc.vector.memset(ones_sb[0:1, :], 1.0)
        ident = cpool.tile([128, 128], FP32, name="ident", tag="ident")
        make_identity(nc, ident[:, :])

        # ---- reduction accumulators ----
        se_acc = accpool.tile([128, R_TILES], FP32, name="se_acc", tag="se_acc")
        si_acc = accpool.tile([128, R_TILES], FP32, name="si_acc", tag="si_acc")
        if SE_ROWS % 128 != 0:
            # rows past SE_ROWS in the last reduce tile are never written by
            # the reduce; zero them so the scratch tail holds no garbage
            nc.vector.memset(se_acc[:, R_TILES - 1 : R_TILES], 0.0)
            nc.vector.memset(si_acc[:, R_TILES - 1 : R_TILES], 0.0)

        reduced = 0  # reduce tiles emitted so far
        stored = 0  # scratch columns stored so far

        def emit_reduce(i):
            r0 = 128 * i
            nr = min(128, SE_ROWS - r0)
            se_t = sepool.tile([128, E_COLS], FP32, name="se_t", tag="se_t")
            first = nc.gpsimd.dma_start(out=se_t[:nr, :], in_=se_h[r0 : r0 + nr, :])
            nc.vector.reduce_sum(se_acc[:nr, i : i + 1], se_t[:nr, :],
                                 axis=mybir.AxisListType.X)
            si_t = sepool.tile([128, I_COLS], FP32, name="si_t", tag="si_t")
            nc.gpsimd.dma_start(out=si_t[:nr, :], in_=si_h[r0 : r0 + nr, :])
            nc.vector.reduce_sum(si_acc[:nr, i : i + 1], si_t[:nr, :],
                                 axis=mybir.AxisListType.X)
            return first

        def emit_store(a, b):
            # PE-transpose fp32 accumulator cols [a,b) to [w,128], cast to
            # bf16 on ScalarE, store contiguously to the scratch vector
            w = b - a
            for nm, acc, scr in (("se", se_acc, sse_h), ("si", si_acc, ssi_h)):
                tr_t = ptrp.tile([16, 128], FP32, name=f"{nm}tr", tag="tr")
                nc.tensor.transpose(tr_t[:w, :], acc[:, a:b], ident[:, :])
                st_t = stpool.tile([16, 128], BF, name=f"{nm}st", tag=f"{nm}st")
                nc.scalar.activation(st_t[:w, :], tr_t[:w, :],
                                     mybir.ActivationFunctionType.Copy)
                nc.scalar.dma_start(
                    out=bass.AP(scr, 128 * a, [[128, w], [1, 128]]),
                    in_=st_t[:w, :],
                )

        # ---- main loop over timestep block pairs ----
        # Blocks are processed in pairs, interleaved at layer granularity:
        # L1(a) L1(b) L2(a) L2(b) ... so the PE never waits for the ScalarE
        # PSUM evacuation of the previous layer (it runs under the other
        # block's matmuls) and stays HAM-warm.  One wide Hankel tile per
        # pair serves both blocks and all K-chunks as column slices.
        # The synaptic-conv contribution is fully decoupled from the MLP:
        # the feed-forward part (W4 h3 + b4) is evacuated to an SBUF "ff"
        # buffer with the pipeline, and the conv matmuls for pair p execute
        # two pairs later -- by then the bulk reduce stream has long
        # produced their scratch operands, so they never stall the PE.
        pairs = [tuple(b for b in (i, i + 1) if b < NB) for i in range(0, NB, 2)]
        NP = len(pairs)
        pair_blks = []
        needs = []
        for pi, pair in enumerate(pairs):
            blks = []
            off = 0
            for b in pair:
                nt = min(BLK, T_PAD - BLK * b)
                blks.append((BLK * b, nt, off))
                off += nt
            pair_blks.append((blks, off))
            lt0, lnt, _ = blks[-1]
            needs.append(R_TILES if pi == NP - 1 else
                         min(R_TILES, _ceil_div(lt0 + lnt + T_NO - 1, 128)))

        synh_tiles = {}
        ff_tiles = {}

        def emit_synh(pi):
            t0p = BLK * pairs[pi][0]
            tot = pair_blks[pi][1]
            synh = {}
            for nm, scr in (("se", sse_h), ("si", ssi_h)):
                t = hkpool.tile([128, 2 * BLK + SY_W], BF, name=f"{nm}h",
                                tag=f"{nm}h")
                nc.sync.dma_start(
                    out=t[:, : tot + SY_W],
                    in_=bass.AP(scr, t0p, [[1, 128], [1, tot + SY_W]]),
                )
                synh[nm] = t
            synh_tiles[pi] = synh

        def emit_stores_until(tgt):
            nonlocal stored
            while stored < tgt:
                emit_store(stored, min(tgt, stored + 16))
                stored = min(tgt, stored + 16)

        def emit_reduces_until(tgt, gate=None):
            nonlocal reduced
            while reduced < tgt:
                first = emit_reduce(reduced)
                if reduced == 0 and gate is not None:
                    # let the startup-critical weight/hankel loads win the
                    # fabric before the bulk stream starts
                    add_dep_helper(first.ins, gate.ins, sync=True,
                                   reason="gate bulk stream on startup loads")
                reduced += 1

        def emit_conv(pi):
            # conv matmuls + ff add + tanh + store for a pair whose ff and
            # scratch hankel operands were produced pairs ago
            blks, tot = pair_blks[pi]
            synh = synh_tiles.pop(pi)
            ffs = ff_tiles.pop(pi)
            for si_, (bt0, nt, coff) in enumerate(blks):
                ps4 = ps4p.tile([SUB, BLK], FP32, name="ps4c", tag="ps4")
                first = True
                for nm, k_sb in (("se", ek_sb), ("si", ik_sb)):
                    for j, (o, pk) in enumerate(k_syn):
                        last = nm == "si" and j == len(k_syn) - 1
                        nc.tensor.matmul(
                            ps4[:, :nt], k_sb[j][:pk, :],
                            synh[nm][:pk, coff + 128 * j : coff + 128 * j + nt],
                            start=first, stop=last,
                        )
                        first = False
                sum_sb = opool.tile([SUB, BLK], FP32, name="sum_sb", tag="sum_sb")
                nc.vector.tensor_add(sum_sb[:, :nt], ps4[:, :nt], ffs[si_][:, :nt])
                out_sb = opool.tile([SUB, BLK], FP32, name="out_sb", tag="out_sb")
                nc.scalar.activation(out_sb[:, :nt], sum_sb[:, :nt],
                                     mybir.ActivationFunctionType.Tanh)
                nc.sync.dma_start(out=out_h[:, bt0 : bt0 + nt], in_=out_sb[:, :nt])

        for pi, pair in enumerate(pairs):
            blks, tot = pair_blks[pi]
            t0p = BLK * pair[0]

            vh = hkpool.tile([128, 2 * BLK + VH_W], BF, name="vh", tag="vh")
            vh_dma = nc.sync.dma_start(
                out=vh[:, : tot + VH_W],
                in_=bass.AP(v_h, t0p, [[1, 128], [1, tot + VH_W]]),
            )
            if pi == 0:
                emit_reduces_until(needs[0], gate=vh_dma)

            # layers 1..3 (hid on PSUM partitions), block-pair interleaved
            h_prev = {}
            for lidx, (k_list, w_sb, bias_t) in enumerate((
                (k_win, w1t_sb, bias_sb["b1"]),
                (k_hid, w2t_sb, bias_sb["b2"]),
                (k_hid, w3t_sb, bias_sb["b3"]),
            )):
                for si_, (bt0, nt, coff) in enumerate(blks):
                    outs = []
                    for mc, (mo, nm_) in enumerate(m_hid):
                        ps = psmm.tile([128, BLK], FP32, name="ps", tag="ps")
                        for kc, (ko, pk) in enumerate(k_list):
                            if lidx == 0:
                                rhs = vh[:pk, coff + 128 * kc : coff + 128 * kc + nt]
                            else:
                                rhs = h_prev[si_][kc][:pk, :nt]
                            nc.tensor.matmul(
                                ps[:nm_, :nt],
                                w_sb[kc][:pk, mo : mo + nm_],
                                rhs,
                                start=(kc == 0),
                                stop=(kc == len(k_list) - 1),
                            )
                        h_t = hpool.tile([128, BLK], BF, name=f"h{lidx}_{mc}",
                                         tag=f"h{lidx}_{mc}")
                        nc.scalar.activation(
                            h_t[:nm_, :nt], ps[:nm_, :nt],
                            mybir.ActivationFunctionType.Lrelu,
                            bias=bias_t[:nm_, mc : mc + 1], alpha=0.01,
                        )
                        outs.append(h_t)
                    h_prev[si_] = outs

            # feed-forward part of layer 4: ff = W4 h3 + b4 -> SBUF
            ffs = []
            for si_, (bt0, nt, coff) in enumerate(blks):
                h3 = h_prev[si_]
                ps4 = ps4p.tile([SUB, BLK], FP32, name="ps4", tag="ps4")
                nc.tensor.matmul(ps4[:, :nt], b4_sb[0:1, :], ones_sb[0:1, :nt],
                                 start=True, stop=False)
                for kc, (ko, pk) in enumerate(k_hid):
                    nc.tensor.matmul(ps4[:, :nt], w4t_sb[kc][:pk, :],
                                     h3[kc][:pk, :nt], start=False,
                                     stop=(kc == len(k_hid) - 1))
                ff_t = opool.tile([SUB, BLK], FP32, name="ff_sb", tag="ff_sb",
                                  bufs=6)
                nc.scalar.activation(ff_t[:, :nt], ps4[:, :nt],
                                     mybir.ActivationFunctionType.Copy)
                ffs.append(ff_t)
            ff_tiles[pi] = ffs

            # this pair's scratch stores + hankel reloads (consumed by the
            # conv two pairs later); transposes land in the PE queue here
            emit_stores_until(needs[pi])
            emit_synh(pi)

            if pi >= 2:
                emit_conv(pi - 2)

            # bulk reduce loads one pair ahead
            emit_reduces_until(needs[min(pi + 1, NP - 1)])

        for pi in range(max(0, NP - 2), NP):
            emit_conv(pi)

    nc.compile()
    return nc


def kernel(V, S_e, S_i, W1, b1, W2, b2, W3, b3, W4, b4, W_syn, Tau_syn, Delta_syn):
    V = np.asarray(V, np.float32)
    S_e = np.ascontiguousarray(np.asarray(S_e, np.float32))
    S_i = np.ascontiguousarray(np.asarray(S_i, np.float32))
    T = V.shape[0]
    assert T % N_CORES == 0
    T_LOC = T // N_CORES
    T_PAD = _ceil_div(T_LOC, 128) * 128
    SE_ROWS = T_NO - 1 + T_LOC
    V_LEN = T_PAD + WIN - 1 + 128
    HID = W1.shape[0]
    SUB = W4.shape[0]

    # ---- tiny host-side prep (layout/dtype only + 20x200 conv kernels) ----
    W_syn = np.asarray(W_syn, np.float32)
    Tau_syn = np.asarray(Tau_syn, np.float32)
    Delta_syn = np.asarray(Delta_syn, np.float32)
    t_raw = np.arange(T_NO, dtype=np.float32)[None, :]
    t_e = np.maximum(t_raw - Delta_syn[:, 0:1], 0.0)
    t_i = np.maximum(t_raw - Delta_syn[:, 1:2], 0.0)
    tt_e = t_e / Tau_syn[:, 0:1] ** 2
    tt_i = t_i / Tau_syn[:, 1:2] ** 2
    e_kern = tt_e * np.exp(-tt_e) * W_syn[:, 0:1] ** 2
    i_kern = -(tt_i * np.exp(-tt_i)) * W_syn[:, 1:2] ** 2
    ekm = np.ascontiguousarray(e_kern[:, ::-1].T).astype(BF16)  # [T_NO, SUB]
    ikm = np.ascontiguousarray(i_kern[:, ::-1].T).astype(BF16)

    def pack_rows(mat, nch):
        # [R, C] -> [128, nch*C]: chunk kc rows at columns [C*kc, C*(kc+1))
        r, c = mat.shape
        out = np.zeros((128, nch * c), np.float32)
        for kc in range(nch):
            rows = mat[128 * kc : min(r, 128 * (kc + 1))]
            out[: rows.shape[0], c * kc : c * kc + c] = rows
        return out

    w1t = np.asarray(W1, np.float32).T
    w2t = np.asarray(W2, np.float32).T
    w3t = np.asarray(W3, np.float32).T
    w4t = np.asarray(W4, np.float32).T
    n_kw, n_kh, n_ks = _ceil_div(WIN, 128), _ceil_div(HID, 128), _ceil_div(T_NO, 128)
    spk = np.concatenate(
        [pack_rows(w4t, n_kh), pack_rows(ekm.astype(np.float32), n_ks),
         pack_rows(ikm.astype(np.float32), n_ks)], 1)
    bpk = np.concatenate(
        [pack_rows(np.asarray(b, np.float32)[:, None], n_kh).reshape(128, n_kh)
         for b in (b1, b2, b3)], 1)
    wd = {
        "w1t": pack_rows(w1t, n_kw).astype(BF16),
        "w2t": pack_rows(w2t, n_kh).astype(BF16),
        "w3t": pack_rows(w3t, n_kh).astype(BF16),
        "bpk": np.ascontiguousarray(bpk, np.float32),
        "b4": np.asarray(b4, np.float32).astype(BF16),
        "spk": spk.astype(BF16),
    }

    vg = np.zeros(T_NO - 1 + T + WIN + 128 + T_PAD - T_LOC, np.float32)
    vg[T_NO - 1 : T_NO - 1 + T] = V
    vg = vg.astype(BF16)

    halo = T_NO - 1
    ez = np.zeros((halo, S_e.shape[1]), np.float32)
    iz = np.zeros((halo, S_i.shape[1]), np.float32)
    in_maps = []
    for m in range(N_CORES):
        r0 = m * T_LOC
        if m == 0:
            se_m = np.concatenate([ez, S_e[:T_LOC]], 0)
            si_m = np.concatenate([iz, S_i[:T_LOC]], 0)
        else:
            se_m = S_e[r0 - halo : r0 + T_LOC]
            si_m = S_i[r0 - halo : r0 + T_LOC]
        in_maps.append(
            {"se": se_m, "si": si_m, "v": vg[r0 : r0 + V_LEN], **wd}
        )

    nc = _build(T_PAD, SE_ROWS, S_e.shape[1], S_i.shape[1], HID, SUB)
    trace = os.environ.get("CC_TRACE") == "1"
    res = run_bass_kernel_spmd(nc, in_maps, list(range(N_CORES)), trace=trace)
    LAST["exec_time_ns"] = res.exec_time_ns
    LAST["results"] = res
    out = np.concatenate(
        [res.results[m]["out"][:, :T_LOC].T for m in range(N_CORES)], 0
    )
    return np.ascontiguousarray(out.astype(np.float32))
